# revision 14
# baseline (speedup 1.0000x reference)
"""Trainium2 Bass kernel for CognitionNetwork (GNN message passing + LSTM attention).

Contract: kernel(**inputs) takes FULL inputs, returns FULL [2048, 400] q_star.
Shards 2048 conversations contiguously across 8 NeuronCores (256 segments each);
each block of 32 segments owns T_pad 128-node tiles (host re-layout).

v2 design (vs v0 per-tile gather):
  - attention scores e come from block-level matmuls contracting FEATURES:
    weights = per-block Q^T (reused across the block's tiles), rhs = a
    feature-major fp16 copy of x. The segment mask is folded into 33 extra
    "features" (indicator rows * 100 on both sides, ones row * -100), so
    e_aug = e + 100*onehot - 100 and exp(e_aug) is already the masked,
    unnormalized attention weight (off-segment entries underflow to 0).
  - exp runs on the scalar engine straight out of PSUM into a bf16 tile;
    per-tile PE transposes flip it node-major; the r matmul streams a bf16
    node-major x copy (ones column appended -> denominator for free).
  - all matmul operands are 16-bit (fp16 for e/LSTM, bf16 for r/phase0):
    1 cycle/row at any output width; fp32 masters kept for h/c/r state.
"""

import os
from contextlib import ExitStack

import ml_dtypes
import numpy as np

import concourse.bass as bass
import concourse.bacc as bacc
import concourse.tile as tile
from concourse import mybir
from concourse.bass_utils import run_bass_kernel_spmd

CORES = 8
B = 2048
F = 200
FW = 201              # node-major x tile width: 200 feats + ones col
SEG_PER_CORE = B // CORES   # 256
BS = 32               # segments per block
BLOCKS = SEG_PER_CORE // BS  # 8
STEPS = 3
KAUG = F + BS + 1     # 233 feature rows incl mask aug
K2 = KAUG - 128       # 105 rows in chunk 2

TRACE = bool(int(os.environ.get("KERNEL_TRACE", "0")))
LAST_RESULT = None
_PROG_CACHE = {}


def _build_program(T_pad: int, nsteps: int = STEPS) -> bass.Bass:
    NT = BLOCKS * T_pad          # node tiles per core
    XFW = NT * 128               # feature-major x width (nodes)
    BW = T_pad * 128             # nodes per block

    nc = bacc.Bacc("TRN2", target_bir_lowering=False, debug=False)
    f32 = mybir.dt.float32
    f32r = mybir.dt.float32r
    f16 = mybir.dt.float16
    bf16 = mybir.dt.bfloat16
    AF = mybir.ActivationFunctionType

    xf1_d = nc.dram_tensor("xf1", [128, XFW], f16, kind="ExternalInput").ap()
    xf2_d = nc.dram_tensor("xf2", [K2, XFW], f16, kind="ExternalInput").ap()
    cwt_d = nc.dram_tensor("cwt", [128, NT * BS], f16, kind="ExternalInput").ap()
    xp_d = nc.dram_tensor("xp", [128, NT * FW], f16, kind="ExternalInput").ap()
    qs0t_d = nc.dram_tensor("qs0t", [401, 256], f16, kind="ExternalInput").ap()
    w0_d = nc.dram_tensor("w0", [634, 800], f16, kind="ExternalInput").ap()
    wc_d = nc.dram_tensor("wc", [434, 800], f16, kind="ExternalInput").ap()
    qc2c_d = nc.dram_tensor("qc2c", [BS + 1, 256], f16, kind="ExternalInput").ap()
    ones_d = nc.dram_tensor("onesr", [1, 256], f16, kind="ExternalInput").ap()
    idf_d = nc.dram_tensor("idf", [128, 128], f32r, kind="ExternalInput").ap()
    idb_d = nc.dram_tensor("idb", [128, 128], bf16, kind="ExternalInput").ap()
    qout_d = nc.dram_tensor("qout", [256, 400], f32, kind="ExternalOutput").ap()

    with tile.TileContext(nc) as tc:
        with ExitStack() as ctx:
            res = ctx.enter_context(tc.tile_pool(name="res", bufs=1))
            state = ctx.enter_context(tc.tile_pool(name="state", bufs=1))
            eap = ctx.enter_context(tc.tile_pool(name="eap", bufs=2))
            xpp = ctx.enter_context(tc.tile_pool(name="xpp", bufs=3))
            eanp = ctx.enter_context(tc.tile_pool(name="eanp", bufs=2))
            sbt = ctx.enter_context(tc.tile_pool(name="sbt", bufs=2))
            psE = ctx.enter_context(tc.tile_pool(name="psE", bufs=2, space="PSUM"))
            psG = ctx.enter_context(tc.tile_pool(name="psG", bufs=2, space="PSUM"))
            psT = ctx.enter_context(tc.tile_pool(name="psT", bufs=2, space="PSUM"))
            psR = ctx.enter_context(tc.tile_pool(name="psR", bufs=2, space="PSUM"))

            # ---------------- resident loads ----------------
            idf = res.tile([128, 128], f32r)
            nc.sync.dma_start(idf[:], idf_d[:])
            idb = res.tile([128, 128], bf16)
            nc.sync.dma_start(idb[:], idb_d[:])

            cwt_sb = res.tile([128, NT * BS], f16)
            xnm_sb = res.tile([128, NT * FW], bf16)
            xf1_sb = res.tile([128, XFW], f16)
            xf2_sb = res.tile([K2, XFW], f16)

            # transposed-input chunks: Q1/Q2 (h^T + mask const), R1/R2 (r^T + ones)
            Q1 = res.tile([128, 256], f16, tag="Q1", name="Q1")
            Q2 = res.tile([K2, 256], f16, tag="Q2", name="Q2")
            nc.sync.dma_start(Q2[72:K2, :], qc2c_d[:])
            R1 = res.tile([128, 256], f16, tag="R1", name="R1")
            R2 = res.tile([73, 256], f16, tag="R2", name="R2")
            nc.sync.dma_start(R2[72:73, :], ones_d[:])

            # fp32 state masters (seg-major, two 128-partition halves)
            h_sb = [state.tile([128, F], f32r, tag=f"h{i}", name=f"h{i}") for i in range(2)]
            c_sb = [state.tile([128, F], f32, tag=f"c{i}", name=f"c{i}") for i in range(2)]
            r_sb = [state.tile([128, F], f32r, tag=f"r{i}", name=f"r{i}") for i in range(2)]
            for i in range(2):
                nc.vector.memset(c_sb[i][:], 0.0)

            # LSTM weights: step0 chunks E0..E3,F0,F1 ; steps>=1 chunks D0..D3
            wE = []
            for k, o in zip([128, 128, 128, 17, 128, K2], [0, 128, 256, 384, 401, 529]):
                t = res.tile([k, 800], f16, tag=f"wE{o}", name=f"wE{o}")
                nc.sync.dma_start(t[:], w0_d[o : o + k, :])
                wE.append(t)
            # step-0 LSTM input chunks (q_star0^T from host)
            qsE = []
            for k, o in zip([128, 128, 128, 17], [0, 128, 256, 384]):
                t = res.tile([k, 256], f16, tag=f"qsE{o}", name=f"qsE{o}")
                nc.sync.dma_start(t[:], qs0t_d[o : o + k, :])
                qsE.append(t)

            # ---------------- phase 0: h0 = segment_sum(cos * x) ----------------
            # quad-stacked; streams fp16 x (with ones col) per block, casting it
            # into the resident bf16 node-major copy as it goes
            for q in range(2):
                h0ps = psR.tile([128, F], f32, tag="rblk")
                for a in range(4):
                    g = 4 * q + a
                    nc.sync.dma_start(
                        cwt_sb[:, g * T_pad * BS : (g + 1) * T_pad * BS],
                        cwt_d[:, g * T_pad * BS : (g + 1) * T_pad * BS],
                    )
                    xpt = xpp.tile([128, T_pad * FW], f16, tag="xp")
                    nc.sync.dma_start(xpt[:], xp_d[:, g * T_pad * FW : (g + 1) * T_pad * FW])
                    for i in range(T_pad):
                        t = g * T_pad + i
                        nc.tensor.matmul(
                            h0ps[32 * a : 32 * a + 32, :],
                            lhsT=cwt_sb[:, t * BS : (t + 1) * BS],
                            rhs=xpt[:, i * FW : i * FW + F],
                            start=(i == 0),
                            stop=(i == T_pad - 1),
                            tile_position=(0, 32 * a),
                        )
                    nc.vector.tensor_copy(
                        xnm_sb[:, g * T_pad * FW : (g + 1) * T_pad * FW], xpt[:]
                    )
                nc.vector.tensor_copy(h_sb[q][:], h0ps[:])
            # bulk x loads (emitted after phase0 so its stream wins the queues)
            wD = []
            for k, o in zip([128, K2, 128, 73], [0, 128, 233, 361]):
                t = res.tile([k, 800], f16, tag=f"wD{o}", name=f"wD{o}")
                nc.sync.dma_start(t[:], wc_d[o : o + k, :])
                wD.append(t)

            for g in range(BLOCKS):
                nc.sync.dma_start(xf1_sb[:, g * BW : (g + 1) * BW], xf1_d[:, g * BW : (g + 1) * BW])
                nc.sync.dma_start(xf2_sb[:, g * BW : (g + 1) * BW], xf2_d[:, g * BW : (g + 1) * BW])

            def emit_hT(src_halves, dst1, dst2, halves=(0, 1)):
                """transpose seg-major [128,200] f32r halves into fp16 feat-major
                chunks: dst1[:, co:co+128] rows 0..127, dst2[0:72, ...] rows 128..199."""
                for half in halves:
                    src = src_halves[half]
                    co = 128 * half
                    t1 = psT.tile([128, 128], f32r, tag="tp")
                    nc.tensor.transpose(t1[:], src[:, 0:128], idf[:])
                    nc.scalar.activation(dst1[:, co : co + 128], t1[:].bitcast(f32), AF.Copy)
                    t2 = psT.tile([72, 128], f32r, tag="tp")
                    nc.tensor.transpose(t2[:], src[:, 128:200], idf[:])
                    nc.scalar.activation(dst2[0:72, co : co + 128], t2[:].bitcast(f32), AF.Copy)

            emit_hT(h_sb, Q1, Q2)

            # ---------------- steps ----------------
            NCH = (BW + 511) // 512  # 512-col e-matmul chunks per block

            def emit_e(q):
                """e_aug matmuls + exp for 4 stacked blocks -> EA [128, BW] bf16."""
                ea = eap.tile([128, BW], bf16, tag="ea", name=f"ea")
                for k in range(NCH):
                    c0 = k * 512
                    cw = min(512, BW - c0)
                    pe = psE.tile([128, 512], f32, tag="pe")
                    for a in range(4):
                        g = 4 * q + a
                        nc.tensor.matmul(
                            pe[32 * a : 32 * a + 32, 0:cw],
                            lhsT=Q1[:, BS * g : BS * (g + 1)],
                            rhs=xf1_sb[:, g * BW + c0 : g * BW + c0 + cw],
                            start=True,
                            stop=False,
                            tile_position=(0, 32 * a),
                        )
                        nc.tensor.matmul(
                            pe[32 * a : 32 * a + 32, 0:cw],
                            lhsT=Q2[0:K2, BS * g : BS * (g + 1)],
                            rhs=xf2_sb[0:K2, g * BW + c0 : g * BW + c0 + cw],
                            start=False,
                            stop=True,
                            tile_position=(0, 32 * a),
                        )
                    nc.scalar.activation(ea[:, c0 : c0 + cw], pe[:, 0:cw], AF.Exp)
                return ea

            def emit_attn_tail(q, ea):
                """transpose EA node-major (4 tiles/instr), r matmuls, normalize."""
                rps = psR.tile([128, F + 1], f32, tag="rblk")
                ean_prev = None
                for i in range(T_pad):
                    tp = psT.tile([128, 128], bf16, tag="tp")
                    nc.tensor.transpose(tp[:], ea[:, 128 * i : 128 * i + 128], idb[:])
                    ean = eanp.tile([128, 128], bf16, tag="ean")
                    if i % 2 == 0:
                        nc.vector.tensor_copy(ean[:], tp[:])
                    else:
                        nc.scalar.activation(ean[:], tp[:], AF.Copy)
                    if ean_prev is not None:
                        _emit_r(q, i - 1, ean_prev, rps)
                    ean_prev = ean
                _emit_r(q, T_pad - 1, ean_prev, rps)
                dinv = sbt.tile([128, 1], f32, tag="dinv")
                nc.vector.reciprocal(dinv[:], rps[:, F : F + 1])
                nc.vector.tensor_scalar_mul(r_sb[q][:], rps[:, 0:F], dinv[:])

            def _emit_r(q, i, ean, rps):
                for a in range(4):
                    t = (4 * q + a) * T_pad + i
                    nc.tensor.matmul(
                        rps[32 * a : 32 * a + 32, :],
                        lhsT=ean[:, 32 * a : 32 * a + 32],
                        rhs=xnm_sb[:, t * FW : t * FW + F + 1],
                        start=(i == 0),
                        stop=(i == T_pad - 1),
                        tile_position=(0, 32 * a),
                    )

            for s in range(nsteps):
                # ---- LSTM cell (seg-major halves) ----
                if s == 0:
                    chunks = list(zip(qsE, [128, 128, 128, 17])) + [(Q1, 128), (Q2, K2)]
                    wts = wE
                else:
                    chunks = [(Q1, 128), (Q2, K2), (R1, 128), (R2, 73)]
                    wts = wD
                def lstm_half(half):
                    co = 128 * half
                    acts = {}
                    for part in range(2):
                        ps = psG.tile([128, 400], f32, tag="gates")
                        nch = len(chunks)
                        for ci, (ctile, kdim) in enumerate(chunks):
                            nc.tensor.matmul(
                                ps[:],
                                lhsT=ctile[0:kdim, co : co + 128],
                                rhs=wts[ci][0:kdim, 400 * part : 400 * part + 400],
                                start=(ci == 0),
                                stop=(ci == nch - 1),
                            )
                        if part == 0:
                            si = sbt.tile([128, F], f32, tag="si")
                            nc.scalar.activation(si[:], ps[:, 0:F], AF.Sigmoid)
                            sf = sbt.tile([128, F], f32, tag="sf")
                            nc.scalar.activation(sf[:], ps[:, F:400], AF.Sigmoid)
                            acts["i"], acts["f"] = si, sf
                        else:
                            tg = sbt.tile([128, F], f32, tag="tg")
                            nc.scalar.activation(tg[:], ps[:, 0:F], AF.Tanh)
                            so = sbt.tile([128, F], f32, tag="so")
                            nc.scalar.activation(so[:], ps[:, F:400], AF.Sigmoid)
                            acts["g"], acts["o"] = tg, so
                    ch = c_sb[half]
                    tmp = sbt.tile([128, F], f32, tag="tmp")
                    nc.vector.tensor_mul(tmp[:], acts["f"][:], ch[:])
                    nc.vector.tensor_mul(ch[:], acts["i"][:], acts["g"][:])
                    nc.vector.tensor_add(ch[:], tmp[:], ch[:])
                    tct = sbt.tile([128, F], f32, tag="tct")
                    nc.scalar.activation(tct[:], ch[:], AF.Tanh)
                    nc.vector.tensor_mul(h_sb[half][:], acts["o"][:], tct[:])

                lstm_half(0)
                lstm_half(1)

                # ---- per-half h^T then e-matmuls: attention starts while the
                # other half's LSTM tail still runs on scalar/vector ----
                emit_hT(h_sb, Q1, Q2, halves=(0,))
                ea0 = emit_e(0)
                emit_hT(h_sb, Q1, Q2, halves=(1,))
                ea1 = emit_e(1)
                emit_attn_tail(0, ea0)
                emit_attn_tail(1, ea1)
                if s < nsteps - 1:
                    emit_hT(r_sb, R1, R2)

            # ---------------- output: q_star = [h | r] ----------------
            for half in range(2):
                ro = 128 * half
                nc.sync.dma_start(qout_d[ro : ro + 128, 0:F], h_sb[half][:].bitcast(f32))
                if nsteps > 0:
                    nc.sync.dma_start(qout_d[ro : ro + 128, F : 2 * F], r_sb[half][:].bitcast(f32))

    nc.compile()
    return nc


def _get_program(T_pad: int) -> bass.Bass:
    nsteps = int(os.environ.get("KERNEL_NSTEPS", str(STEPS)))
    key = (T_pad, nsteps)
    if key not in _PROG_CACHE:
        _PROG_CACHE[key] = _build_program(T_pad, nsteps)
    return _PROG_CACHE[key]


def make_in_maps(x, batch, cos_coef, q_star, W_ih, W_hh, b_ih, b_hh):
    """Host-side shard + re-layout. Returns (in_maps, T_pad)."""
    x = np.ascontiguousarray(np.asarray(x, dtype=np.float32))
    batch = np.asarray(batch).astype(np.int64)
    cos = np.asarray(cos_coef, dtype=np.float32)
    qs = np.asarray(q_star, dtype=np.float32)
    W_ih = np.asarray(W_ih, dtype=np.float32)
    W_hh = np.asarray(W_hh, dtype=np.float32)
    bsum = (np.asarray(b_ih, dtype=np.float32) + np.asarray(b_hh, dtype=np.float32))

    counts = np.bincount(batch, minlength=B)
    starts = np.zeros(B + 1, dtype=np.int64)
    starts[1:] = np.cumsum(counts)
    blk_counts = counts.reshape(-1, BS).sum(axis=1)
    T_pad = int(max(1, -(-blk_counts.max() // 128)))
    NT = BLOCKS * T_pad
    BW = T_pad * 128

    bf = ml_dtypes.bfloat16

    # LSTM weight stacks (fp16)
    W_ihT = W_ih.T  # [400, 800]
    W_hhT = W_hh.T  # [200, 800]
    w0 = np.concatenate(
        [W_ihT, bsum[None, :], W_hhT, np.zeros((BS + 1, 800), np.float32)], axis=0
    ).astype(np.float16)  # [634, 800]; rows 529.. = W_hhT[128:200] + aug zeros
    WcT = W_ihT[:F] + W_hhT          # [200, 800]
    WrT = W_ihT[F:]                  # [200, 800]
    wc = np.concatenate(
        [WcT[0:128], WcT[128:200], np.zeros((BS + 1, 800), np.float32),
         WrT[0:128], WrT[128:200], bsum[None, :]], axis=0
    ).astype(np.float16)             # [434, 800]

    qc2c = np.zeros((BS + 1, 256), np.float16)
    qc2c[0:BS] = np.tile(100.0 * np.eye(BS, dtype=np.float32), (1, BLOCKS))
    qc2c[BS] = -100.0

    in_maps = []
    for c in range(CORES):
        seg0 = c * SEG_PER_CORE
        xf = np.zeros((KAUG, NT * 128), dtype=np.float16)
        cwt = np.zeros((128, NT * BS), dtype=np.float16)
        xp = np.zeros((128, NT * FW), dtype=np.float16)
        for g in range(BLOCKS):
            sa = seg0 + g * BS
            n0, n1 = int(starts[sa]), int(starts[sa + BS])
            cnt = n1 - n0
            js = (batch[n0:n1] - sa).astype(np.int64)

            xb = np.zeros((BW, FW), dtype=np.float32)
            xb[:cnt, :F] = x[n0:n1]
            xb[:cnt, F] = 1.0
            xp[:, g * T_pad * FW : (g + 1) * T_pad * FW] = (
                xb.reshape(T_pad, 128, FW).transpose(1, 0, 2).reshape(128, T_pad * FW)
            ).astype(np.float16)

            xfb = np.zeros((KAUG, BW), dtype=np.float32)
            xfb[0:F, :cnt] = x[n0:n1].T
            xfb[F + js, np.arange(cnt)] = 1.0
            xfb[F + BS, :] = 1.0
            xf[:, g * BW : (g + 1) * BW] = xfb.astype(np.float16)

            wb = np.zeros((BW, BS), dtype=np.float32)
            wb[np.arange(cnt), js] = cos[n0:n1]
            cwt[:, g * T_pad * BS : (g + 1) * T_pad * BS] = (
                wb.reshape(T_pad, 128, BS).transpose(1, 0, 2).reshape(128, T_pad * BS)
            ).astype(np.float16)

        qs0t = np.ones((401, 256), dtype=np.float16)
        qs0t[0:400] = qs[seg0 : seg0 + SEG_PER_CORE].T.astype(np.float16)
        in_maps.append(
            {
                "xf1": np.ascontiguousarray(xf[0:128]),
                "xf2": np.ascontiguousarray(xf[128:KAUG]),
                "cwt": cwt,
                "xp": xp,
                "qs0t": qs0t,
                "w0": w0,
                "wc": wc,
                "qc2c": qc2c,
                "onesr": np.ones((1, 256), np.float16),
                "idf": np.eye(128, dtype=np.float32),
                "idb": np.eye(128, dtype=np.float32).astype(bf),
            }
        )
    return in_maps, T_pad


def kernel(x, batch, cos_coef, q_star, W_ih, W_hh, b_ih, b_hh):
    global LAST_RESULT
    in_maps, T_pad = make_in_maps(
        x, batch, cos_coef, q_star, W_ih, W_hh, b_ih, b_hh
    )
    nc = _get_program(T_pad)
    res = run_bass_kernel_spmd(nc, in_maps, list(range(CORES)), trace=TRACE)
    LAST_RESULT = res
    out = np.zeros((B, 2 * F), dtype=np.float32)
    for c in range(CORES):
        out[c * SEG_PER_CORE : (c + 1) * SEG_PER_CORE] = res.results[c]["qout"]
    return out


# revision 15
# speedup vs baseline: 1.0785x; 1.0785x over previous
"""Trainium2 Bass kernel for CognitionNetwork (GNN message passing + LSTM attention).

Contract: kernel(**inputs) takes FULL inputs, returns FULL [2048, 400] q_star.
Shards 2048 conversations contiguously across 8 NeuronCores (256 segments each);
each block of 32 segments owns T_pad 128-node tiles (host re-layout).

v2 design (vs v0 per-tile gather):
  - attention scores e come from block-level matmuls contracting FEATURES:
    weights = per-block Q^T (reused across the block's tiles), rhs = a
    feature-major fp16 copy of x. The segment mask is folded into 33 extra
    "features" (indicator rows * 100 on both sides, ones row * -100), so
    e_aug = e + 100*onehot - 100 and exp(e_aug) is already the masked,
    unnormalized attention weight (off-segment entries underflow to 0).
  - exp runs on the scalar engine straight out of PSUM into a bf16 tile;
    per-tile PE transposes flip it node-major; the r matmul streams a bf16
    node-major x copy (ones column appended -> denominator for free).
  - all matmul operands are 16-bit (fp16 for e/LSTM, bf16 for r/phase0):
    1 cycle/row at any output width; fp32 masters kept for h/c/r state.
"""

import os
from contextlib import ExitStack

import ml_dtypes
import numpy as np

import concourse.bass as bass
import concourse.bacc as bacc
import concourse.tile as tile
from concourse import mybir
from concourse.bass_utils import run_bass_kernel_spmd

CORES = 8
B = 2048
F = 200
FW = 201              # node-major x tile width: 200 feats + ones col
SEG_PER_CORE = B // CORES   # 256
BS = 32               # segments per block
BLOCKS = SEG_PER_CORE // BS  # 8
STEPS = 3
KAUG = F + BS + 1     # 233 feature rows incl mask aug
K2 = KAUG - 128       # 105 rows in chunk 2

TRACE = bool(int(os.environ.get("KERNEL_TRACE", "0")))
LAST_RESULT = None
_PROG_CACHE = {}


def _build_program(T_pad: int, nsteps: int = STEPS) -> bass.Bass:
    NT = BLOCKS * T_pad          # node tiles per core
    XFW = NT * 128               # feature-major x width (nodes)
    BW = T_pad * 128             # nodes per block

    nc = bacc.Bacc("TRN2", target_bir_lowering=False, debug=False)
    f32 = mybir.dt.float32
    f32r = mybir.dt.float32r
    f16 = mybir.dt.float16
    bf16 = mybir.dt.bfloat16
    AF = mybir.ActivationFunctionType

    xf1_d = nc.dram_tensor("xf1", [128, XFW], f16, kind="ExternalInput").ap()
    xf2_d = nc.dram_tensor("xf2", [K2, XFW], f16, kind="ExternalInput").ap()
    cwt_d = nc.dram_tensor("cwt", [128, NT * BS], f16, kind="ExternalInput").ap()
    xp_d = nc.dram_tensor("xp", [128, NT * FW], f16, kind="ExternalInput").ap()
    qs0t_d = nc.dram_tensor("qs0t", [401, 256], f16, kind="ExternalInput").ap()
    w0_d = nc.dram_tensor("w0", [634, 800], f16, kind="ExternalInput").ap()
    wc_d = nc.dram_tensor("wc", [434, 800], f16, kind="ExternalInput").ap()
    qc2c_d = nc.dram_tensor("qc2c", [BS + 1, 256], f16, kind="ExternalInput").ap()
    ones_d = nc.dram_tensor("onesr", [1, 256], f16, kind="ExternalInput").ap()
    idf_d = nc.dram_tensor("idf", [128, 128], f32r, kind="ExternalInput").ap()
    idb_d = nc.dram_tensor("idb", [128, 128], bf16, kind="ExternalInput").ap()
    qout_d = nc.dram_tensor("qout", [256, 400], f32, kind="ExternalOutput").ap()

    with tile.TileContext(nc) as tc:
        with ExitStack() as ctx:
            res = ctx.enter_context(tc.tile_pool(name="res", bufs=1))
            state = ctx.enter_context(tc.tile_pool(name="state", bufs=1))
            eap = ctx.enter_context(tc.tile_pool(name="eap", bufs=2))
            xpp = ctx.enter_context(tc.tile_pool(name="xpp", bufs=3))
            eanp = ctx.enter_context(tc.tile_pool(name="eanp", bufs=2))
            sbt = ctx.enter_context(tc.tile_pool(name="sbt", bufs=2))
            psE = ctx.enter_context(tc.tile_pool(name="psE", bufs=2, space="PSUM"))
            psG = ctx.enter_context(tc.tile_pool(name="psG", bufs=2, space="PSUM"))
            psT = ctx.enter_context(tc.tile_pool(name="psT", bufs=2, space="PSUM"))
            psR = ctx.enter_context(tc.tile_pool(name="psR", bufs=2, space="PSUM"))

            # ---------------- resident loads ----------------
            idf = res.tile([128, 128], f32r)
            nc.sync.dma_start(idf[:], idf_d[:])
            idb = res.tile([128, 128], bf16)
            nc.sync.dma_start(idb[:], idb_d[:])

            cwt_sb = res.tile([128, NT * BS], f16)
            xnm_sb = res.tile([128, NT * FW], bf16)
            xf1_sb = res.tile([128, XFW], f16)
            xf2_sb = res.tile([K2, XFW], f16)

            # transposed-input chunks: Q1/Q2 (h^T + mask const), R1/R2 (r^T + ones)
            Q1 = res.tile([128, 256], f16, tag="Q1", name="Q1")
            Q2 = res.tile([K2, 256], f16, tag="Q2", name="Q2")
            nc.sync.dma_start(Q2[72:K2, :], qc2c_d[:])
            R1 = res.tile([128, 256], f16, tag="R1", name="R1")
            R2 = res.tile([73, 256], f16, tag="R2", name="R2")
            nc.sync.dma_start(R2[72:73, :], ones_d[:])

            # fp32 state masters (seg-major, two 128-partition halves)
            h_sb = [state.tile([128, F], f32r, tag=f"h{i}", name=f"h{i}") for i in range(2)]
            c_sb = [state.tile([128, F], f32, tag=f"c{i}", name=f"c{i}") for i in range(2)]
            r_sb = [state.tile([128, F], f32r, tag=f"r{i}", name=f"r{i}") for i in range(2)]
            for i in range(2):
                nc.vector.memset(c_sb[i][:], 0.0)

            # LSTM weights: step0 chunks E0..E3,F0,F1 ; steps>=1 chunks D0..D3
            wE = []
            for k, o in zip([128, 128, 128, 17, 128, K2], [0, 128, 256, 384, 401, 529]):
                t = res.tile([k, 800], f16, tag=f"wE{o}", name=f"wE{o}")
                nc.sync.dma_start(t[:], w0_d[o : o + k, :])
                wE.append(t)
            # step-0 LSTM input chunks (q_star0^T from host)
            qsE = []
            for k, o in zip([128, 128, 128, 17], [0, 128, 256, 384]):
                t = res.tile([k, 256], f16, tag=f"qsE{o}", name=f"qsE{o}")
                nc.sync.dma_start(t[:], qs0t_d[o : o + k, :])
                qsE.append(t)

            # ---------------- phase 0: h0 = segment_sum(cos * x) ----------------
            # quad-stacked; streams fp16 x (with ones col) per block, casting it
            # into the resident bf16 node-major copy as it goes
            for q in range(2):
                h0ps = psR.tile([128, F], f32, tag="rblk")
                for a in range(4):
                    g = 4 * q + a
                    nc.sync.dma_start(
                        cwt_sb[:, g * T_pad * BS : (g + 1) * T_pad * BS],
                        cwt_d[:, g * T_pad * BS : (g + 1) * T_pad * BS],
                    )
                    xpt = xpp.tile([128, T_pad * FW], f16, tag="xp")
                    nc.sync.dma_start(xpt[:], xp_d[:, g * T_pad * FW : (g + 1) * T_pad * FW])
                    for i in range(T_pad):
                        t = g * T_pad + i
                        nc.tensor.matmul(
                            h0ps[32 * a : 32 * a + 32, :],
                            lhsT=cwt_sb[:, t * BS : (t + 1) * BS],
                            rhs=xpt[:, i * FW : i * FW + F],
                            start=(i == 0),
                            stop=(i == T_pad - 1),
                            tile_position=(0, 32 * a),
                        )
                    nc.vector.tensor_copy(
                        xnm_sb[:, g * T_pad * FW : (g + 1) * T_pad * FW], xpt[:]
                    )
                nc.vector.tensor_copy(h_sb[q][:], h0ps[:])
            # bulk x loads (emitted after phase0 so its stream wins the queues)
            wD = []
            for k, o in zip([128, K2, 128, 73], [0, 128, 233, 361]):
                t = res.tile([k, 800], f16, tag=f"wD{o}", name=f"wD{o}")
                nc.sync.dma_start(t[:], wc_d[o : o + k, :])
                wD.append(t)

            for g in range(BLOCKS):
                nc.sync.dma_start(xf1_sb[:, g * BW : (g + 1) * BW], xf1_d[:, g * BW : (g + 1) * BW])
                nc.sync.dma_start(xf2_sb[:, g * BW : (g + 1) * BW], xf2_d[:, g * BW : (g + 1) * BW])

            def emit_hT(src_halves, dst1, dst2, halves=(0, 1)):
                """transpose seg-major [128,200] f32r halves into fp16 feat-major
                chunks: dst1[:, co:co+128] rows 0..127, dst2[0:72, ...] rows 128..199."""
                for half in halves:
                    src = src_halves[half]
                    co = 128 * half
                    t1 = psT.tile([128, 128], f32r, tag="tp")
                    nc.tensor.transpose(t1[:], src[:, 0:128], idf[:])
                    nc.vector.tensor_copy(dst1[:, co : co + 128], t1[:].bitcast(f32))
                    t2 = psT.tile([72, 128], f32r, tag="tp")
                    nc.tensor.transpose(t2[:], src[:, 128:200], idf[:])
                    nc.vector.tensor_copy(dst2[0:72, co : co + 128], t2[:].bitcast(f32))

            emit_hT(h_sb, Q1, Q2)

            # ---------------- steps ----------------
            NCH = (BW + 511) // 512  # 512-col e-matmul chunks per block

            def emit_e(q):
                """e_aug matmuls + exp for 4 stacked blocks -> EA [128, BW] bf16."""
                ea = eap.tile([128, BW], bf16, tag="ea", name=f"ea")
                for k in range(NCH):
                    c0 = k * 512
                    cw = min(512, BW - c0)
                    pe = psE.tile([128, 512], f32, tag="pe")
                    for a in range(4):
                        g = 4 * q + a
                        nc.tensor.matmul(
                            pe[32 * a : 32 * a + 32, 0:cw],
                            lhsT=Q1[:, BS * g : BS * (g + 1)],
                            rhs=xf1_sb[:, g * BW + c0 : g * BW + c0 + cw],
                            start=True,
                            stop=False,
                            tile_position=(0, 32 * a),
                        )
                        nc.tensor.matmul(
                            pe[32 * a : 32 * a + 32, 0:cw],
                            lhsT=Q2[0:K2, BS * g : BS * (g + 1)],
                            rhs=xf2_sb[0:K2, g * BW + c0 : g * BW + c0 + cw],
                            start=False,
                            stop=True,
                            tile_position=(0, 32 * a),
                        )
                    nc.scalar.activation(ea[:, c0 : c0 + cw], pe[:, 0:cw], AF.Exp)
                return ea

            def emit_attn_tail(q, ea):
                """transpose EA node-major (4 tiles/instr), r matmuls, normalize."""
                rps = psR.tile([128, F + 1], f32, tag="rblk")
                ean_prev = None
                for i in range(T_pad):
                    tp = psT.tile([128, 128], bf16, tag="tp")
                    nc.tensor.transpose(tp[:], ea[:, 128 * i : 128 * i + 128], idb[:])
                    ean = eanp.tile([128, 128], bf16, tag="ean")
                    nc.vector.tensor_copy(ean[:], tp[:])
                    if ean_prev is not None:
                        _emit_r(q, i - 1, ean_prev, rps)
                    ean_prev = ean
                _emit_r(q, T_pad - 1, ean_prev, rps)
                dinv = sbt.tile([128, 1], f32, tag="dinv")
                nc.vector.reciprocal(dinv[:], rps[:, F : F + 1])
                nc.vector.tensor_scalar_mul(r_sb[q][:], rps[:, 0:F], dinv[:])

            def _emit_r(q, i, ean, rps):
                for a in range(4):
                    t = (4 * q + a) * T_pad + i
                    nc.tensor.matmul(
                        rps[32 * a : 32 * a + 32, :],
                        lhsT=ean[:, 32 * a : 32 * a + 32],
                        rhs=xnm_sb[:, t * FW : t * FW + F + 1],
                        start=(i == 0),
                        stop=(i == T_pad - 1),
                        tile_position=(0, 32 * a),
                    )

            for s in range(nsteps):
                # ---- LSTM cell (seg-major halves) ----
                if s == 0:
                    chunks = list(zip(qsE, [128, 128, 128, 17])) + [(Q1, 128), (Q2, K2)]
                    wts = wE
                else:
                    chunks = [(Q1, 128), (Q2, K2), (R1, 128), (R2, 73)]
                    wts = wD
                def lstm_half(half):
                    co = 128 * half
                    acts = {}
                    for part in range(2):
                        ps = psG.tile([128, 400], f32, tag="gates")
                        nch = len(chunks)
                        for ci, (ctile, kdim) in enumerate(chunks):
                            nc.tensor.matmul(
                                ps[:],
                                lhsT=ctile[0:kdim, co : co + 128],
                                rhs=wts[ci][0:kdim, 400 * part : 400 * part + 400],
                                start=(ci == 0),
                                stop=(ci == nch - 1),
                            )
                        if part == 0:
                            si = sbt.tile([128, F], f32, tag="si")
                            nc.scalar.activation(si[:], ps[:, 0:F], AF.Sigmoid)
                            sf = sbt.tile([128, F], f32, tag="sf")
                            nc.scalar.activation(sf[:], ps[:, F:400], AF.Sigmoid)
                            acts["i"], acts["f"] = si, sf
                        else:
                            tg = sbt.tile([128, F], f32, tag="tg")
                            nc.scalar.activation(tg[:], ps[:, 0:F], AF.Tanh)
                            so = sbt.tile([128, F], f32, tag="so")
                            nc.scalar.activation(so[:], ps[:, F:400], AF.Sigmoid)
                            acts["g"], acts["o"] = tg, so
                    ch = c_sb[half]
                    tmp = sbt.tile([128, F], f32, tag="tmp")
                    nc.vector.tensor_mul(tmp[:], acts["f"][:], ch[:])
                    nc.vector.tensor_mul(ch[:], acts["i"][:], acts["g"][:])
                    nc.vector.tensor_add(ch[:], tmp[:], ch[:])
                    tct = sbt.tile([128, F], f32, tag="tct")
                    nc.scalar.activation(tct[:], ch[:], AF.Tanh)
                    nc.vector.tensor_mul(h_sb[half][:], acts["o"][:], tct[:])

                lstm_half(0)
                lstm_half(1)

                # ---- per-half h^T then e-matmuls: attention starts while the
                # other half's LSTM tail still runs on scalar/vector ----
                emit_hT(h_sb, Q1, Q2, halves=(0,))
                ea0 = emit_e(0)
                emit_hT(h_sb, Q1, Q2, halves=(1,))
                ea1 = emit_e(1)
                emit_attn_tail(0, ea0)
                emit_attn_tail(1, ea1)
                if s < nsteps - 1:
                    emit_hT(r_sb, R1, R2)

            # ---------------- output: q_star = [h | r] ----------------
            for half in range(2):
                ro = 128 * half
                nc.sync.dma_start(qout_d[ro : ro + 128, 0:F], h_sb[half][:].bitcast(f32))
                if nsteps > 0:
                    nc.sync.dma_start(qout_d[ro : ro + 128, F : 2 * F], r_sb[half][:].bitcast(f32))

    nc.compile()
    return nc


def _get_program(T_pad: int) -> bass.Bass:
    nsteps = int(os.environ.get("KERNEL_NSTEPS", str(STEPS)))
    key = (T_pad, nsteps)
    if key not in _PROG_CACHE:
        _PROG_CACHE[key] = _build_program(T_pad, nsteps)
    return _PROG_CACHE[key]


def make_in_maps(x, batch, cos_coef, q_star, W_ih, W_hh, b_ih, b_hh):
    """Host-side shard + re-layout. Returns (in_maps, T_pad)."""
    x = np.ascontiguousarray(np.asarray(x, dtype=np.float32))
    batch = np.asarray(batch).astype(np.int64)
    cos = np.asarray(cos_coef, dtype=np.float32)
    qs = np.asarray(q_star, dtype=np.float32)
    W_ih = np.asarray(W_ih, dtype=np.float32)
    W_hh = np.asarray(W_hh, dtype=np.float32)
    bsum = (np.asarray(b_ih, dtype=np.float32) + np.asarray(b_hh, dtype=np.float32))

    counts = np.bincount(batch, minlength=B)
    starts = np.zeros(B + 1, dtype=np.int64)
    starts[1:] = np.cumsum(counts)
    blk_counts = counts.reshape(-1, BS).sum(axis=1)
    T_pad = int(max(1, -(-blk_counts.max() // 128)))
    NT = BLOCKS * T_pad
    BW = T_pad * 128

    bf = ml_dtypes.bfloat16

    # LSTM weight stacks (fp16)
    W_ihT = W_ih.T  # [400, 800]
    W_hhT = W_hh.T  # [200, 800]
    w0 = np.concatenate(
        [W_ihT, bsum[None, :], W_hhT, np.zeros((BS + 1, 800), np.float32)], axis=0
    ).astype(np.float16)  # [634, 800]; rows 529.. = W_hhT[128:200] + aug zeros
    WcT = W_ihT[:F] + W_hhT          # [200, 800]
    WrT = W_ihT[F:]                  # [200, 800]
    wc = np.concatenate(
        [WcT[0:128], WcT[128:200], np.zeros((BS + 1, 800), np.float32),
         WrT[0:128], WrT[128:200], bsum[None, :]], axis=0
    ).astype(np.float16)             # [434, 800]

    qc2c = np.zeros((BS + 1, 256), np.float16)
    qc2c[0:BS] = np.tile(100.0 * np.eye(BS, dtype=np.float32), (1, BLOCKS))
    qc2c[BS] = -100.0

    in_maps = []
    for c in range(CORES):
        seg0 = c * SEG_PER_CORE
        xf = np.zeros((KAUG, NT * 128), dtype=np.float16)
        cwt = np.zeros((128, NT * BS), dtype=np.float16)
        xp = np.zeros((128, NT * FW), dtype=np.float16)
        for g in range(BLOCKS):
            sa = seg0 + g * BS
            n0, n1 = int(starts[sa]), int(starts[sa + BS])
            cnt = n1 - n0
            js = (batch[n0:n1] - sa).astype(np.int64)

            xb = np.zeros((BW, FW), dtype=np.float32)
            xb[:cnt, :F] = x[n0:n1]
            xb[:cnt, F] = 1.0
            xp[:, g * T_pad * FW : (g + 1) * T_pad * FW] = (
                xb.reshape(T_pad, 128, FW).transpose(1, 0, 2).reshape(128, T_pad * FW)
            ).astype(np.float16)

            xfb = np.zeros((KAUG, BW), dtype=np.float32)
            xfb[0:F, :cnt] = x[n0:n1].T
            xfb[F + js, np.arange(cnt)] = 1.0
            xfb[F + BS, :] = 1.0
            xf[:, g * BW : (g + 1) * BW] = xfb.astype(np.float16)

            wb = np.zeros((BW, BS), dtype=np.float32)
            wb[np.arange(cnt), js] = cos[n0:n1]
            cwt[:, g * T_pad * BS : (g + 1) * T_pad * BS] = (
                wb.reshape(T_pad, 128, BS).transpose(1, 0, 2).reshape(128, T_pad * BS)
            ).astype(np.float16)

        qs0t = np.ones((401, 256), dtype=np.float16)
        qs0t[0:400] = qs[seg0 : seg0 + SEG_PER_CORE].T.astype(np.float16)
        in_maps.append(
            {
                "xf1": np.ascontiguousarray(xf[0:128]),
                "xf2": np.ascontiguousarray(xf[128:KAUG]),
                "cwt": cwt,
                "xp": xp,
                "qs0t": qs0t,
                "w0": w0,
                "wc": wc,
                "qc2c": qc2c,
                "onesr": np.ones((1, 256), np.float16),
                "idf": np.eye(128, dtype=np.float32),
                "idb": np.eye(128, dtype=np.float32).astype(bf),
            }
        )
    return in_maps, T_pad


def kernel(x, batch, cos_coef, q_star, W_ih, W_hh, b_ih, b_hh):
    global LAST_RESULT
    in_maps, T_pad = make_in_maps(
        x, batch, cos_coef, q_star, W_ih, W_hh, b_ih, b_hh
    )
    nc = _get_program(T_pad)
    res = run_bass_kernel_spmd(nc, in_maps, list(range(CORES)), trace=TRACE)
    LAST_RESULT = res
    out = np.zeros((B, 2 * F), dtype=np.float32)
    for c in range(CORES):
        out[c * SEG_PER_CORE : (c + 1) * SEG_PER_CORE] = res.results[c]["qout"]
    return out


# revision 16
# speedup vs baseline: 1.0840x; 1.0051x over previous
"""Trainium2 Bass kernel for CognitionNetwork (GNN message passing + LSTM attention).

Contract: kernel(**inputs) takes FULL inputs, returns FULL [2048, 400] q_star.
Shards 2048 conversations contiguously across 8 NeuronCores (256 segments each);
each block of 32 segments owns T_pad 128-node tiles (host re-layout).

v2 design (vs v0 per-tile gather):
  - attention scores e come from block-level matmuls contracting FEATURES:
    weights = per-block Q^T (reused across the block's tiles), rhs = a
    feature-major fp16 copy of x. The segment mask is folded into 33 extra
    "features" (indicator rows * 100 on both sides, ones row * -100), so
    e_aug = e + 100*onehot - 100 and exp(e_aug) is already the masked,
    unnormalized attention weight (off-segment entries underflow to 0).
  - exp runs on the scalar engine straight out of PSUM into a bf16 tile;
    per-tile PE transposes flip it node-major; the r matmul streams a bf16
    node-major x copy (ones column appended -> denominator for free).
  - all matmul operands are 16-bit (fp16 for e/LSTM, bf16 for r/phase0):
    1 cycle/row at any output width; fp32 masters kept for h/c/r state.
"""

import os
from contextlib import ExitStack

import ml_dtypes
import numpy as np

import concourse.bass as bass
import concourse.bacc as bacc
import concourse.tile as tile
from concourse import mybir
from concourse.bass_utils import run_bass_kernel_spmd

CORES = 8
B = 2048
F = 200
FW = 201              # node-major x tile width: 200 feats + ones col
SEG_PER_CORE = B // CORES   # 256
BS = 32               # segments per block
BLOCKS = SEG_PER_CORE // BS  # 8
STEPS = 3
KAUG = F + BS + 1     # 233 feature rows incl mask aug
K2 = KAUG - 128       # 105 rows in chunk 2

TRACE = bool(int(os.environ.get("KERNEL_TRACE", "0")))
LAST_RESULT = None
_PROG_CACHE = {}


def _build_program(T_pad: int, nsteps: int = STEPS) -> bass.Bass:
    NT = BLOCKS * T_pad          # node tiles per core
    XFW = NT * 128               # feature-major x width (nodes)
    BW = T_pad * 128             # nodes per block

    nc = bacc.Bacc("TRN2", target_bir_lowering=False, debug=False)
    f32 = mybir.dt.float32
    f32r = mybir.dt.float32r
    f16 = mybir.dt.float16
    bf16 = mybir.dt.bfloat16
    AF = mybir.ActivationFunctionType

    xf1_d = nc.dram_tensor("xf1", [128, XFW], f16, kind="ExternalInput").ap()
    xf2_d = nc.dram_tensor("xf2", [K2, XFW], f16, kind="ExternalInput").ap()
    cwt_d = nc.dram_tensor("cwt", [128, NT * BS], f16, kind="ExternalInput").ap()
    xp_d = nc.dram_tensor("xp", [128, NT * FW], f16, kind="ExternalInput").ap()
    qs0t_d = nc.dram_tensor("qs0t", [401, 256], f16, kind="ExternalInput").ap()
    w0_d = nc.dram_tensor("w0", [634, 800], f16, kind="ExternalInput").ap()
    wc_d = nc.dram_tensor("wc", [434, 800], f16, kind="ExternalInput").ap()
    qc2c_d = nc.dram_tensor("qc2c", [BS + 1, 256], f16, kind="ExternalInput").ap()
    ones_d = nc.dram_tensor("onesr", [1, 256], f16, kind="ExternalInput").ap()
    idf_d = nc.dram_tensor("idf", [128, 128], f32r, kind="ExternalInput").ap()
    idb_d = nc.dram_tensor("idb", [128, 128], bf16, kind="ExternalInput").ap()
    qout_d = nc.dram_tensor("qout", [256, 400], f32, kind="ExternalOutput").ap()

    with tile.TileContext(nc) as tc:
        with ExitStack() as ctx:
            res = ctx.enter_context(tc.tile_pool(name="res", bufs=1))
            state = ctx.enter_context(tc.tile_pool(name="state", bufs=1))
            eap = ctx.enter_context(tc.tile_pool(name="eap", bufs=2))
            xpp = ctx.enter_context(tc.tile_pool(name="xpp", bufs=3))
            eanp = ctx.enter_context(tc.tile_pool(name="eanp", bufs=2))
            sbt = ctx.enter_context(tc.tile_pool(name="sbt", bufs=2))
            psE = ctx.enter_context(tc.tile_pool(name="psE", bufs=2, space="PSUM"))
            psG = ctx.enter_context(tc.tile_pool(name="psG", bufs=2, space="PSUM"))
            psT = ctx.enter_context(tc.tile_pool(name="psT", bufs=2, space="PSUM"))
            psR = ctx.enter_context(tc.tile_pool(name="psR", bufs=2, space="PSUM"))

            # ---------------- resident loads ----------------
            idf = res.tile([128, 128], f32r)
            nc.sync.dma_start(idf[:], idf_d[:])
            idb = res.tile([128, 128], bf16)
            nc.sync.dma_start(idb[:], idb_d[:])

            cwt_sb = res.tile([128, NT * BS], f16)
            xnm_sb = res.tile([128, NT * FW], bf16)
            xf1_sb = res.tile([128, XFW], f16)
            xf2_sb = res.tile([K2, XFW], f16)

            # transposed-input chunks: Q1/Q2 (h^T + mask const), R1/R2 (r^T + ones)
            Q1 = res.tile([128, 256], f16, tag="Q1", name="Q1")
            Q2 = res.tile([K2, 256], f16, tag="Q2", name="Q2")
            nc.sync.dma_start(Q2[72:K2, :], qc2c_d[:])
            R1 = res.tile([128, 256], f16, tag="R1", name="R1")
            R2 = res.tile([73, 256], f16, tag="R2", name="R2")
            nc.sync.dma_start(R2[72:73, :], ones_d[:])

            # fp32 state masters (seg-major, two 128-partition halves)
            h_sb = [state.tile([128, F], f32r, tag=f"h{i}", name=f"h{i}") for i in range(2)]
            c_sb = [state.tile([128, F], f32, tag=f"c{i}", name=f"c{i}") for i in range(2)]
            r_sb = [state.tile([128, F], f32r, tag=f"r{i}", name=f"r{i}") for i in range(2)]
            for i in range(2):
                nc.vector.memset(c_sb[i][:], 0.0)

            # LSTM weights: step0 chunks E0..E3,F0,F1 ; steps>=1 chunks D0..D3
            wE = []
            for k, o in zip([128, 128, 128, 17, 128, K2], [0, 128, 256, 384, 401, 529]):
                t = res.tile([k, 800], f16, tag=f"wE{o}", name=f"wE{o}")
                nc.sync.dma_start(t[:], w0_d[o : o + k, :])
                wE.append(t)
            # step-0 LSTM input chunks (q_star0^T from host)
            qsE = []
            for k, o in zip([128, 128, 128, 17], [0, 128, 256, 384]):
                t = res.tile([k, 256], f16, tag=f"qsE{o}", name=f"qsE{o}")
                nc.sync.dma_start(t[:], qs0t_d[o : o + k, :])
                qsE.append(t)

            # ---------------- phase 0: h0 = segment_sum(cos * x) ----------------
            # quad-stacked; streams fp16 x (with ones col) per block, casting it
            # into the resident bf16 node-major copy as it goes
            for q in range(2):
                h0ps = psR.tile([128, F], f32, tag="rblk")
                for a in range(4):
                    g = 4 * q + a
                    nc.sync.dma_start(
                        cwt_sb[:, g * T_pad * BS : (g + 1) * T_pad * BS],
                        cwt_d[:, g * T_pad * BS : (g + 1) * T_pad * BS],
                    )
                    xpt = xpp.tile([128, T_pad * FW], f16, tag="xp")
                    nc.sync.dma_start(xpt[:], xp_d[:, g * T_pad * FW : (g + 1) * T_pad * FW])
                    for i in range(T_pad):
                        t = g * T_pad + i
                        nc.tensor.matmul(
                            h0ps[32 * a : 32 * a + 32, :],
                            lhsT=cwt_sb[:, t * BS : (t + 1) * BS],
                            rhs=xpt[:, i * FW : i * FW + F],
                            start=(i == 0),
                            stop=(i == T_pad - 1),
                            tile_position=(0, 32 * a),
                        )
                    nc.vector.tensor_copy(
                        xnm_sb[:, g * T_pad * FW : (g + 1) * T_pad * FW], xpt[:]
                    )
                nc.vector.tensor_copy(h_sb[q][:], h0ps[:])
            # bulk x loads (emitted after phase0 so its stream wins the queues)
            wD = []
            for k, o in zip([128, K2, 128, 73], [0, 128, 233, 361]):
                t = res.tile([k, 800], f16, tag=f"wD{o}", name=f"wD{o}")
                nc.sync.dma_start(t[:], wc_d[o : o + k, :])
                wD.append(t)

            for g in range(BLOCKS):
                nc.sync.dma_start(xf1_sb[:, g * BW : (g + 1) * BW], xf1_d[:, g * BW : (g + 1) * BW])
                nc.sync.dma_start(xf2_sb[:, g * BW : (g + 1) * BW], xf2_d[:, g * BW : (g + 1) * BW])

            def emit_hT(src_halves, dst1, dst2, halves=(0, 1)):
                """transpose seg-major [128,200] f32r halves into fp16 feat-major
                chunks: dst1[:, co:co+128] rows 0..127, dst2[0:72, ...] rows 128..199."""
                for half in halves:
                    src = src_halves[half]
                    co = 128 * half
                    t1 = psT.tile([128, 128], f32r, tag="tp")
                    nc.tensor.transpose(t1[:], src[:, 0:128], idf[:])
                    nc.vector.tensor_copy(dst1[:, co : co + 128], t1[:].bitcast(f32))
                    t2 = psT.tile([72, 128], f32r, tag="tp")
                    nc.tensor.transpose(t2[:], src[:, 128:200], idf[:])
                    nc.vector.tensor_copy(dst2[0:72, co : co + 128], t2[:].bitcast(f32))

            emit_hT(h_sb, Q1, Q2)

            # ---------------- steps ----------------
            NCH = (BW + 511) // 512  # 512-col e-matmul chunks per block

            def emit_e(q):
                """e_aug matmuls + exp for 4 stacked blocks -> EA [128, BW] bf16."""
                ea = eap.tile([128, BW], bf16, tag="ea", name=f"ea")
                for k in range(NCH):
                    c0 = k * 512
                    cw = min(512, BW - c0)
                    pe = psE.tile([128, 512], f32, tag="pe")
                    for a in range(4):
                        g = 4 * q + a
                        nc.tensor.matmul(
                            pe[32 * a : 32 * a + 32, 0:cw],
                            lhsT=Q1[:, BS * g : BS * (g + 1)],
                            rhs=xf1_sb[:, g * BW + c0 : g * BW + c0 + cw],
                            start=True,
                            stop=False,
                            tile_position=(0, 32 * a),
                        )
                        nc.tensor.matmul(
                            pe[32 * a : 32 * a + 32, 0:cw],
                            lhsT=Q2[0:K2, BS * g : BS * (g + 1)],
                            rhs=xf2_sb[0:K2, g * BW + c0 : g * BW + c0 + cw],
                            start=False,
                            stop=True,
                            tile_position=(0, 32 * a),
                        )
                    nc.scalar.activation(ea[:, c0 : c0 + cw], pe[:, 0:cw], AF.Exp)
                return ea

            def emit_attn_tail(q, ea):
                """transpose EA node-major (4 tiles/instr), r matmuls, normalize."""
                rps = psR.tile([128, F + 1], f32, tag="rblk")
                ean_prev = None
                for i in range(T_pad):
                    tp = psT.tile([128, 128], bf16, tag="tp")
                    nc.tensor.transpose(tp[:], ea[:, 128 * i : 128 * i + 128], idb[:])
                    ean = eanp.tile([128, 128], bf16, tag="ean")
                    nc.vector.tensor_copy(ean[:], tp[:])
                    if ean_prev is not None:
                        _emit_r(q, i - 1, ean_prev, rps)
                    ean_prev = ean
                _emit_r(q, T_pad - 1, ean_prev, rps)
                dinv = sbt.tile([128, 1], f32, tag="dinv")
                nc.vector.reciprocal(dinv[:], rps[:, F : F + 1])
                nc.vector.tensor_scalar_mul(r_sb[q][:], rps[:, 0:F], dinv[:])

            def _emit_r(q, i, ean, rps):
                for a in range(4):
                    t = (4 * q + a) * T_pad + i
                    nc.tensor.matmul(
                        rps[32 * a : 32 * a + 32, :],
                        lhsT=ean[:, 32 * a : 32 * a + 32],
                        rhs=xnm_sb[:, t * FW : t * FW + F + 1],
                        start=(i == 0),
                        stop=(i == T_pad - 1),
                        tile_position=(0, 32 * a),
                    )

            for s in range(nsteps):
                # ---- LSTM cell (seg-major halves) ----
                if s == 0:
                    chunks = list(zip(qsE, [128, 128, 128, 17])) + [(Q1, 128), (Q2, K2)]
                    wts = wE
                else:
                    chunks = [(Q1, 128), (Q2, K2), (R1, 128), (R2, 73)]
                    wts = wD
                def lstm_half(half):
                    co = 128 * half
                    acts = {}
                    for part in range(2):
                        ps = psG.tile([128, 400], f32, tag="gates")
                        nch = len(chunks)
                        for ci, (ctile, kdim) in enumerate(chunks):
                            nc.tensor.matmul(
                                ps[:],
                                lhsT=ctile[0:kdim, co : co + 128],
                                rhs=wts[ci][0:kdim, 400 * part : 400 * part + 400],
                                start=(ci == 0),
                                stop=(ci == nch - 1),
                            )
                        if part == 0:
                            si = sbt.tile([128, F], f32, tag="si")
                            nc.scalar.activation(si[:], ps[:, 0:F], AF.Sigmoid)
                            sf = sbt.tile([128, F], f32, tag="sf")
                            nc.scalar.activation(sf[:], ps[:, F:400], AF.Sigmoid)
                            acts["i"], acts["f"] = si, sf
                        else:
                            tg = sbt.tile([128, F], f32, tag="tg")
                            nc.scalar.activation(tg[:], ps[:, 0:F], AF.Tanh)
                            so = sbt.tile([128, F], f32, tag="so")
                            nc.scalar.activation(so[:], ps[:, F:400], AF.Sigmoid)
                            acts["g"], acts["o"] = tg, so
                    ch = c_sb[half]
                    tmp = sbt.tile([128, F], f32, tag="tmp")
                    nc.vector.tensor_mul(tmp[:], acts["f"][:], ch[:])
                    nc.vector.tensor_mul(ch[:], acts["i"][:], acts["g"][:])
                    nc.vector.tensor_add(ch[:], tmp[:], ch[:])
                    tct = sbt.tile([128, F], f32, tag="tct")
                    nc.scalar.activation(tct[:], ch[:], AF.Tanh)
                    nc.vector.tensor_mul(h_sb[half][:], acts["o"][:], tct[:])

                lstm_half(0)
                lstm_half(1)
                if s == nsteps - 1:
                    for half in range(2):
                        nc.sync.dma_start(
                            qout_d[128 * half : 128 * half + 128, 0:F],
                            h_sb[half][:].bitcast(f32),
                        )

                # ---- per-half h^T then e-matmuls: attention starts while the
                # other half's LSTM tail still runs on scalar/vector ----
                emit_hT(h_sb, Q1, Q2, halves=(0,))
                ea0 = emit_e(0)
                emit_hT(h_sb, Q1, Q2, halves=(1,))
                ea1 = emit_e(1)
                emit_attn_tail(0, ea0)
                if s == nsteps - 1:
                    nc.sync.dma_start(qout_d[0:128, F : 2 * F], r_sb[0][:].bitcast(f32))
                emit_attn_tail(1, ea1)
                if s == nsteps - 1:
                    nc.sync.dma_start(qout_d[128:256, F : 2 * F], r_sb[1][:].bitcast(f32))
                if s < nsteps - 1:
                    emit_hT(r_sb, R1, R2)

            if nsteps == 0:
                for half in range(2):
                    nc.sync.dma_start(
                        qout_d[128 * half : 128 * half + 128, 0:F], h_sb[half][:].bitcast(f32)
                    )

    nc.compile()
    return nc


def _get_program(T_pad: int) -> bass.Bass:
    nsteps = int(os.environ.get("KERNEL_NSTEPS", str(STEPS)))
    key = (T_pad, nsteps)
    if key not in _PROG_CACHE:
        _PROG_CACHE[key] = _build_program(T_pad, nsteps)
    return _PROG_CACHE[key]


def make_in_maps(x, batch, cos_coef, q_star, W_ih, W_hh, b_ih, b_hh):
    """Host-side shard + re-layout. Returns (in_maps, T_pad)."""
    x = np.ascontiguousarray(np.asarray(x, dtype=np.float32))
    batch = np.asarray(batch).astype(np.int64)
    cos = np.asarray(cos_coef, dtype=np.float32)
    qs = np.asarray(q_star, dtype=np.float32)
    W_ih = np.asarray(W_ih, dtype=np.float32)
    W_hh = np.asarray(W_hh, dtype=np.float32)
    bsum = (np.asarray(b_ih, dtype=np.float32) + np.asarray(b_hh, dtype=np.float32))

    counts = np.bincount(batch, minlength=B)
    starts = np.zeros(B + 1, dtype=np.int64)
    starts[1:] = np.cumsum(counts)
    blk_counts = counts.reshape(-1, BS).sum(axis=1)
    T_pad = int(max(1, -(-blk_counts.max() // 128)))
    NT = BLOCKS * T_pad
    BW = T_pad * 128

    bf = ml_dtypes.bfloat16

    # LSTM weight stacks (fp16)
    W_ihT = W_ih.T  # [400, 800]
    W_hhT = W_hh.T  # [200, 800]
    w0 = np.concatenate(
        [W_ihT, bsum[None, :], W_hhT, np.zeros((BS + 1, 800), np.float32)], axis=0
    ).astype(np.float16)  # [634, 800]; rows 529.. = W_hhT[128:200] + aug zeros
    WcT = W_ihT[:F] + W_hhT          # [200, 800]
    WrT = W_ihT[F:]                  # [200, 800]
    wc = np.concatenate(
        [WcT[0:128], WcT[128:200], np.zeros((BS + 1, 800), np.float32),
         WrT[0:128], WrT[128:200], bsum[None, :]], axis=0
    ).astype(np.float16)             # [434, 800]

    qc2c = np.zeros((BS + 1, 256), np.float16)
    qc2c[0:BS] = np.tile(100.0 * np.eye(BS, dtype=np.float32), (1, BLOCKS))
    qc2c[BS] = -100.0

    in_maps = []
    for c in range(CORES):
        seg0 = c * SEG_PER_CORE
        xf = np.zeros((KAUG, NT * 128), dtype=np.float16)
        cwt = np.zeros((128, NT * BS), dtype=np.float16)
        xp = np.zeros((128, NT * FW), dtype=np.float16)
        for g in range(BLOCKS):
            sa = seg0 + g * BS
            n0, n1 = int(starts[sa]), int(starts[sa + BS])
            cnt = n1 - n0
            js = (batch[n0:n1] - sa).astype(np.int64)

            xb = np.zeros((BW, FW), dtype=np.float32)
            xb[:cnt, :F] = x[n0:n1]
            xb[:cnt, F] = 1.0
            xp[:, g * T_pad * FW : (g + 1) * T_pad * FW] = (
                xb.reshape(T_pad, 128, FW).transpose(1, 0, 2).reshape(128, T_pad * FW)
            ).astype(np.float16)

            xfb = np.zeros((KAUG, BW), dtype=np.float32)
            xfb[0:F, :cnt] = x[n0:n1].T
            xfb[F + js, np.arange(cnt)] = 1.0
            xfb[F + BS, :] = 1.0
            xf[:, g * BW : (g + 1) * BW] = xfb.astype(np.float16)

            wb = np.zeros((BW, BS), dtype=np.float32)
            wb[np.arange(cnt), js] = cos[n0:n1]
            cwt[:, g * T_pad * BS : (g + 1) * T_pad * BS] = (
                wb.reshape(T_pad, 128, BS).transpose(1, 0, 2).reshape(128, T_pad * BS)
            ).astype(np.float16)

        qs0t = np.ones((401, 256), dtype=np.float16)
        qs0t[0:400] = qs[seg0 : seg0 + SEG_PER_CORE].T.astype(np.float16)
        in_maps.append(
            {
                "xf1": np.ascontiguousarray(xf[0:128]),
                "xf2": np.ascontiguousarray(xf[128:KAUG]),
                "cwt": cwt,
                "xp": xp,
                "qs0t": qs0t,
                "w0": w0,
                "wc": wc,
                "qc2c": qc2c,
                "onesr": np.ones((1, 256), np.float16),
                "idf": np.eye(128, dtype=np.float32),
                "idb": np.eye(128, dtype=np.float32).astype(bf),
            }
        )
    return in_maps, T_pad


def kernel(x, batch, cos_coef, q_star, W_ih, W_hh, b_ih, b_hh):
    global LAST_RESULT
    in_maps, T_pad = make_in_maps(
        x, batch, cos_coef, q_star, W_ih, W_hh, b_ih, b_hh
    )
    nc = _get_program(T_pad)
    res = run_bass_kernel_spmd(nc, in_maps, list(range(CORES)), trace=TRACE)
    LAST_RESULT = res
    out = np.zeros((B, 2 * F), dtype=np.float32)
    for c in range(CORES):
        out[c * SEG_PER_CORE : (c + 1) * SEG_PER_CORE] = res.results[c]["qout"]
    return out


# revision 18
# speedup vs baseline: 1.1748x; 1.0837x over previous
"""Trainium2 Bass kernel for CognitionNetwork (GNN message passing + LSTM attention).

Contract: kernel(**inputs) takes FULL inputs, returns FULL [2048, 400] q_star.
Shards 2048 conversations contiguously across 8 NeuronCores (256 segments each);
each block of 32 segments owns T_pad 128-node tiles (host re-layout).

v2 design (vs v0 per-tile gather):
  - attention scores e come from block-level matmuls contracting FEATURES:
    weights = per-block Q^T (reused across the block's tiles), rhs = a
    feature-major fp16 copy of x. The segment mask is folded into 33 extra
    "features" (indicator rows * 100 on both sides, ones row * -100), so
    e_aug = e + 100*onehot - 100 and exp(e_aug) is already the masked,
    unnormalized attention weight (off-segment entries underflow to 0).
  - exp runs on the scalar engine straight out of PSUM into a bf16 tile;
    per-tile PE transposes flip it node-major; the r matmul streams a bf16
    node-major x copy (ones column appended -> denominator for free).
  - all matmul operands are 16-bit (fp16 for e/LSTM, bf16 for r/phase0):
    1 cycle/row at any output width; fp32 masters kept for h/c/r state.
"""

import os
from contextlib import ExitStack

import ml_dtypes
import numpy as np

import concourse.bass as bass
import concourse.bacc as bacc
import concourse.tile as tile
from concourse import mybir
from concourse.bass_utils import run_bass_kernel_spmd

CORES = 8
B = 2048
F = 200
FW = 201              # node-major x tile width: 200 feats + ones col
SEG_PER_CORE = B // CORES   # 256
BS = 32               # segments per block
BLOCKS = SEG_PER_CORE // BS  # 8
STEPS = 3
KAUG = F + BS + 1     # 233 feature rows incl mask aug
K2 = KAUG - 128       # 105 rows in chunk 2

TRACE = bool(int(os.environ.get("KERNEL_TRACE", "0")))
LAST_RESULT = None
_PROG_CACHE = {}


def _build_program(T_pad: int, nsteps: int = STEPS) -> bass.Bass:
    NT = BLOCKS * T_pad          # node tiles per core
    XFW = NT * 128               # feature-major x width (nodes)
    BW = T_pad * 128             # nodes per block

    nc = bacc.Bacc("TRN2", target_bir_lowering=False, debug=False)
    f32 = mybir.dt.float32
    f32r = mybir.dt.float32r
    f16 = mybir.dt.float16
    bf16 = mybir.dt.bfloat16
    AF = mybir.ActivationFunctionType

    xf1_d = nc.dram_tensor("xf1", [128, XFW], f16, kind="ExternalInput").ap()
    xf2_d = nc.dram_tensor("xf2", [K2, XFW], f16, kind="ExternalInput").ap()
    cwt_d = nc.dram_tensor("cwt", [128, NT * BS], f16, kind="ExternalInput").ap()
    xp_d = nc.dram_tensor("xp", [128, NT * FW], f16, kind="ExternalInput").ap()
    qs0t_d = nc.dram_tensor("qs0t", [401, 256], f16, kind="ExternalInput").ap()
    w0_d = nc.dram_tensor("w0", [634, 800], f16, kind="ExternalInput").ap()
    wc_d = nc.dram_tensor("wc", [434, 800], f16, kind="ExternalInput").ap()
    qc2c_d = nc.dram_tensor("qc2c", [BS + 1, 256], f16, kind="ExternalInput").ap()
    ones_d = nc.dram_tensor("onesr", [1, 256], f16, kind="ExternalInput").ap()
    idf_d = nc.dram_tensor("idf", [128, 128], f32r, kind="ExternalInput").ap()
    idb_d = nc.dram_tensor("idb", [128, 128], bf16, kind="ExternalInput").ap()
    qout_d = nc.dram_tensor("qout", [256, 400], f32, kind="ExternalOutput").ap()

    with tile.TileContext(nc) as tc:
        with ExitStack() as ctx:
            res = ctx.enter_context(tc.tile_pool(name="res", bufs=1))
            state = ctx.enter_context(tc.tile_pool(name="state", bufs=1))
            eap = ctx.enter_context(tc.tile_pool(name="eap", bufs=2))
            xpp = ctx.enter_context(tc.tile_pool(name="xpp", bufs=3))
            eanp = ctx.enter_context(tc.tile_pool(name="eanp", bufs=2))
            sbt = ctx.enter_context(tc.tile_pool(name="sbt", bufs=2))
            psE = ctx.enter_context(tc.tile_pool(name="psE", bufs=2, space="PSUM"))
            psG = ctx.enter_context(tc.tile_pool(name="psG", bufs=2, space="PSUM"))
            psT = ctx.enter_context(tc.tile_pool(name="psT", bufs=2, space="PSUM"))
            psR = ctx.enter_context(tc.tile_pool(name="psR", bufs=2, space="PSUM"))

            # ---------------- resident loads ----------------
            idf = res.tile([128, 128], f32r)
            nc.sync.dma_start(idf[:], idf_d[:])
            idb = res.tile([128, 128], bf16)
            nc.sync.dma_start(idb[:], idb_d[:])

            cwt_sb = res.tile([128, NT * BS], f16)
            xnm_sb = res.tile([128, NT * FW], bf16)
            xf1_sb = res.tile([128, XFW], f16)
            xf2_sb = res.tile([K2, XFW], f16)

            # transposed-input chunks: Q1/Q2 (h^T + mask const), R1/R2 (r^T + ones)
            Q1 = res.tile([128, 256], f16, tag="Q1", name="Q1")
            Q2 = res.tile([K2, 256], f16, tag="Q2", name="Q2")
            nc.sync.dma_start(Q2[72:K2, :], qc2c_d[:])
            R1 = res.tile([128, 256], f16, tag="R1", name="R1")
            R2 = res.tile([73, 256], f16, tag="R2", name="R2")
            nc.sync.dma_start(R2[72:73, :], ones_d[:])

            # fp32 state masters (seg-major, two 128-partition halves)
            h_sb = [state.tile([128, F], f32r, tag=f"h{i}", name=f"h{i}") for i in range(2)]
            c_sb = [state.tile([128, F], f32, tag=f"c{i}", name=f"c{i}") for i in range(2)]
            r_sb = [state.tile([128, F], f32r, tag=f"r{i}", name=f"r{i}") for i in range(2)]
            for i in range(2):
                nc.vector.memset(c_sb[i][:], 0.0)

            # ---------------- phase 0: h0 = segment_sum(cos * x) ----------------
            # quad-stacked; streams fp16 x (with ones col) per block, casting it
            # into the resident bf16 node-major copy as it goes
            for q in range(2):
                h0ps = psR.tile([128, F], f32, tag="rblk")
                for a in range(4):
                    g = 4 * q + a
                    nc.sync.dma_start(
                        cwt_sb[:, g * T_pad * BS : (g + 1) * T_pad * BS],
                        cwt_d[:, g * T_pad * BS : (g + 1) * T_pad * BS],
                    )
                    xpt = xpp.tile([128, T_pad * FW], f16, tag="xp")
                    nc.sync.dma_start(xpt[:], xp_d[:, g * T_pad * FW : (g + 1) * T_pad * FW])
                    for i in range(T_pad):
                        t = g * T_pad + i
                        nc.tensor.matmul(
                            h0ps[32 * a : 32 * a + 32, :],
                            lhsT=cwt_sb[:, t * BS : (t + 1) * BS],
                            rhs=xpt[:, i * FW : i * FW + F],
                            start=(i == 0),
                            stop=(i == T_pad - 1),
                            tile_position=(0, 32 * a),
                        )
                    nc.vector.tensor_copy(
                        xnm_sb[:, g * T_pad * FW : (g + 1) * T_pad * FW], xpt[:]
                    )
                nc.vector.tensor_copy(h_sb[q][:], h0ps[:])

            # remaining loads, in consumption order: LSTM0 weights, then
            # feature-major x for attention, then step>=1 weights
            wE = []
            for k, o in zip([128, 128, 128, 17, 128, K2], [0, 128, 256, 384, 401, 529]):
                t = res.tile([k, 800], f16, tag=f"wE{o}", name=f"wE{o}")
                nc.sync.dma_start(t[:], w0_d[o : o + k, :])
                wE.append(t)
            qsE = []
            for k, o in zip([128, 128, 128, 17], [0, 128, 256, 384]):
                t = res.tile([k, 256], f16, tag=f"qsE{o}", name=f"qsE{o}")
                nc.sync.dma_start(t[:], qs0t_d[o : o + k, :])
                qsE.append(t)
            for g in range(BLOCKS):
                nc.sync.dma_start(xf1_sb[:, g * BW : (g + 1) * BW], xf1_d[:, g * BW : (g + 1) * BW])
                nc.sync.dma_start(xf2_sb[:, g * BW : (g + 1) * BW], xf2_d[:, g * BW : (g + 1) * BW])
            wD = []
            for k, o in zip([128, K2, 128, 73], [0, 128, 233, 361]):
                t = res.tile([k, 800], f16, tag=f"wD{o}", name=f"wD{o}")
                nc.sync.dma_start(t[:], wc_d[o : o + k, :])
                wD.append(t)

            def emit_hT(src_halves, dst1, dst2, halves=(0, 1)):
                """transpose seg-major [128,200] f32r halves into fp16 feat-major
                chunks: dst1[:, co:co+128] rows 0..127, dst2[0:72, ...] rows 128..199."""
                for half in halves:
                    src = src_halves[half]
                    co = 128 * half
                    t1 = psT.tile([128, 128], f32r, tag="tp")
                    nc.tensor.transpose(t1[:], src[:, 0:128], idf[:])
                    nc.vector.tensor_copy(dst1[:, co : co + 128], t1[:].bitcast(f32))
                    t2 = psT.tile([72, 128], f32r, tag="tp")
                    nc.tensor.transpose(t2[:], src[:, 128:200], idf[:])
                    nc.vector.tensor_copy(dst2[0:72, co : co + 128], t2[:].bitcast(f32))

            emit_hT(h_sb, Q1, Q2)

            # ---------------- steps ----------------
            NCH = (BW + 511) // 512  # 512-col e-matmul chunks per block

            def emit_e(q):
                """e_aug matmuls + exp for 4 stacked blocks -> EA [128, BW] bf16."""
                ea = eap.tile([128, BW], bf16, tag="ea", name=f"ea")
                for k in range(NCH):
                    c0 = k * 512
                    cw = min(512, BW - c0)
                    pe = psE.tile([128, 512], f32, tag="pe")
                    for a in range(4):
                        g = 4 * q + a
                        nc.tensor.matmul(
                            pe[32 * a : 32 * a + 32, 0:cw],
                            lhsT=Q1[:, BS * g : BS * (g + 1)],
                            rhs=xf1_sb[:, g * BW + c0 : g * BW + c0 + cw],
                            start=True,
                            stop=False,
                            tile_position=(0, 32 * a),
                        )
                        nc.tensor.matmul(
                            pe[32 * a : 32 * a + 32, 0:cw],
                            lhsT=Q2[0:K2, BS * g : BS * (g + 1)],
                            rhs=xf2_sb[0:K2, g * BW + c0 : g * BW + c0 + cw],
                            start=False,
                            stop=True,
                            tile_position=(0, 32 * a),
                        )
                    nc.scalar.activation(ea[:, c0 : c0 + cw], pe[:, 0:cw], AF.Exp)
                return ea

            def emit_attn_tail(q, ea):
                """transpose EA node-major (4 tiles/instr), r matmuls, normalize."""
                rps = psR.tile([128, F + 1], f32, tag="rblk")
                ean_prev = None
                for i in range(T_pad):
                    tp = psT.tile([128, 128], bf16, tag="tp")
                    nc.tensor.transpose(tp[:], ea[:, 128 * i : 128 * i + 128], idb[:])
                    ean = eanp.tile([128, 128], bf16, tag="ean")
                    nc.vector.tensor_copy(ean[:], tp[:])
                    if ean_prev is not None:
                        _emit_r(q, i - 1, ean_prev, rps)
                    ean_prev = ean
                _emit_r(q, T_pad - 1, ean_prev, rps)
                dinv = sbt.tile([128, 1], f32, tag="dinv")
                nc.vector.reciprocal(dinv[:], rps[:, F : F + 1])
                nc.vector.tensor_scalar_mul(r_sb[q][:], rps[:, 0:F], dinv[:])

            def _emit_r(q, i, ean, rps):
                for a in range(4):
                    t = (4 * q + a) * T_pad + i
                    nc.tensor.matmul(
                        rps[32 * a : 32 * a + 32, :],
                        lhsT=ean[:, 32 * a : 32 * a + 32],
                        rhs=xnm_sb[:, t * FW : t * FW + F + 1],
                        start=(i == 0),
                        stop=(i == T_pad - 1),
                        tile_position=(0, 32 * a),
                    )

            for s in range(nsteps):
                # ---- LSTM cell (seg-major halves) ----
                if s == 0:
                    chunks = list(zip(qsE, [128, 128, 128, 17])) + [(Q1, 128), (Q2, K2)]
                    wts = wE
                else:
                    chunks = [(Q1, 128), (Q2, K2), (R1, 128), (R2, 73)]
                    wts = wD
                def lstm_half(half):
                    co = 128 * half
                    acts = {}
                    for part in range(2):
                        ps = psG.tile([128, 400], f32, tag="gates")
                        nch = len(chunks)
                        for ci, (ctile, kdim) in enumerate(chunks):
                            nc.tensor.matmul(
                                ps[:],
                                lhsT=ctile[0:kdim, co : co + 128],
                                rhs=wts[ci][0:kdim, 400 * part : 400 * part + 400],
                                start=(ci == 0),
                                stop=(ci == nch - 1),
                            )
                        if part == 0:
                            si = sbt.tile([128, F], f32, tag="si")
                            nc.scalar.activation(si[:], ps[:, 0:F], AF.Sigmoid)
                            sf = sbt.tile([128, F], f32, tag="sf")
                            nc.scalar.activation(sf[:], ps[:, F:400], AF.Sigmoid)
                            acts["i"], acts["f"] = si, sf
                        else:
                            tg = sbt.tile([128, F], f32, tag="tg")
                            nc.scalar.activation(tg[:], ps[:, 0:F], AF.Tanh)
                            so = sbt.tile([128, F], f32, tag="so")
                            nc.scalar.activation(so[:], ps[:, F:400], AF.Sigmoid)
                            acts["g"], acts["o"] = tg, so
                    ch = c_sb[half]
                    tmp = sbt.tile([128, F], f32, tag="tmp")
                    nc.vector.tensor_mul(tmp[:], acts["f"][:], ch[:])
                    nc.vector.tensor_mul(ch[:], acts["i"][:], acts["g"][:])
                    nc.vector.tensor_add(ch[:], tmp[:], ch[:])
                    tct = sbt.tile([128, F], f32, tag="tct")
                    nc.scalar.activation(tct[:], ch[:], AF.Tanh)
                    nc.vector.tensor_mul(h_sb[half][:], acts["o"][:], tct[:])

                lstm_half(0)
                lstm_half(1)
                if s == nsteps - 1:
                    for half in range(2):
                        nc.sync.dma_start(
                            qout_d[128 * half : 128 * half + 128, 0:F],
                            h_sb[half][:].bitcast(f32),
                        )

                # ---- per-half h^T then e-matmuls: attention starts while the
                # other half's LSTM tail still runs on scalar/vector ----
                emit_hT(h_sb, Q1, Q2, halves=(0,))
                ea0 = emit_e(0)
                emit_hT(h_sb, Q1, Q2, halves=(1,))
                ea1 = emit_e(1)
                emit_attn_tail(0, ea0)
                if s == nsteps - 1:
                    nc.sync.dma_start(qout_d[0:128, F : 2 * F], r_sb[0][:].bitcast(f32))
                emit_attn_tail(1, ea1)
                if s == nsteps - 1:
                    nc.sync.dma_start(qout_d[128:256, F : 2 * F], r_sb[1][:].bitcast(f32))
                if s < nsteps - 1:
                    emit_hT(r_sb, R1, R2)

            if nsteps == 0:
                for half in range(2):
                    nc.sync.dma_start(
                        qout_d[128 * half : 128 * half + 128, 0:F], h_sb[half][:].bitcast(f32)
                    )

    nc.compile()
    return nc


def _get_program(T_pad: int) -> bass.Bass:
    nsteps = int(os.environ.get("KERNEL_NSTEPS", str(STEPS)))
    key = (T_pad, nsteps)
    if key not in _PROG_CACHE:
        _PROG_CACHE[key] = _build_program(T_pad, nsteps)
    return _PROG_CACHE[key]


def make_in_maps(x, batch, cos_coef, q_star, W_ih, W_hh, b_ih, b_hh):
    """Host-side shard + re-layout. Returns (in_maps, T_pad)."""
    x = np.ascontiguousarray(np.asarray(x, dtype=np.float32))
    batch = np.asarray(batch).astype(np.int64)
    cos = np.asarray(cos_coef, dtype=np.float32)
    qs = np.asarray(q_star, dtype=np.float32)
    W_ih = np.asarray(W_ih, dtype=np.float32)
    W_hh = np.asarray(W_hh, dtype=np.float32)
    bsum = (np.asarray(b_ih, dtype=np.float32) + np.asarray(b_hh, dtype=np.float32))

    counts = np.bincount(batch, minlength=B)
    starts = np.zeros(B + 1, dtype=np.int64)
    starts[1:] = np.cumsum(counts)
    blk_counts = counts.reshape(-1, BS).sum(axis=1)
    T_pad = int(max(1, -(-blk_counts.max() // 128)))
    NT = BLOCKS * T_pad
    BW = T_pad * 128

    bf = ml_dtypes.bfloat16

    # LSTM weight stacks (fp16)
    W_ihT = W_ih.T  # [400, 800]
    W_hhT = W_hh.T  # [200, 800]
    w0 = np.concatenate(
        [W_ihT, bsum[None, :], W_hhT, np.zeros((BS + 1, 800), np.float32)], axis=0
    ).astype(np.float16)  # [634, 800]; rows 529.. = W_hhT[128:200] + aug zeros
    WcT = W_ihT[:F] + W_hhT          # [200, 800]
    WrT = W_ihT[F:]                  # [200, 800]
    wc = np.concatenate(
        [WcT[0:128], WcT[128:200], np.zeros((BS + 1, 800), np.float32),
         WrT[0:128], WrT[128:200], bsum[None, :]], axis=0
    ).astype(np.float16)             # [434, 800]

    qc2c = np.zeros((BS + 1, 256), np.float16)
    qc2c[0:BS] = np.tile(100.0 * np.eye(BS, dtype=np.float32), (1, BLOCKS))
    qc2c[BS] = -100.0

    in_maps = []
    for c in range(CORES):
        seg0 = c * SEG_PER_CORE
        xf = np.zeros((KAUG, NT * 128), dtype=np.float16)
        cwt = np.zeros((128, NT * BS), dtype=np.float16)
        xp = np.zeros((128, NT * FW), dtype=np.float16)
        for g in range(BLOCKS):
            sa = seg0 + g * BS
            n0, n1 = int(starts[sa]), int(starts[sa + BS])
            cnt = n1 - n0
            js = (batch[n0:n1] - sa).astype(np.int64)

            xb = np.zeros((BW, FW), dtype=np.float32)
            xb[:cnt, :F] = x[n0:n1]
            xb[:cnt, F] = 1.0
            xp[:, g * T_pad * FW : (g + 1) * T_pad * FW] = (
                xb.reshape(T_pad, 128, FW).transpose(1, 0, 2).reshape(128, T_pad * FW)
            ).astype(np.float16)

            xfb = np.zeros((KAUG, BW), dtype=np.float32)
            xfb[0:F, :cnt] = x[n0:n1].T
            xfb[F + js, np.arange(cnt)] = 1.0
            xfb[F + BS, :] = 1.0
            xf[:, g * BW : (g + 1) * BW] = xfb.astype(np.float16)

            wb = np.zeros((BW, BS), dtype=np.float32)
            wb[np.arange(cnt), js] = cos[n0:n1]
            cwt[:, g * T_pad * BS : (g + 1) * T_pad * BS] = (
                wb.reshape(T_pad, 128, BS).transpose(1, 0, 2).reshape(128, T_pad * BS)
            ).astype(np.float16)

        qs0t = np.ones((401, 256), dtype=np.float16)
        qs0t[0:400] = qs[seg0 : seg0 + SEG_PER_CORE].T.astype(np.float16)
        in_maps.append(
            {
                "xf1": np.ascontiguousarray(xf[0:128]),
                "xf2": np.ascontiguousarray(xf[128:KAUG]),
                "cwt": cwt,
                "xp": xp,
                "qs0t": qs0t,
                "w0": w0,
                "wc": wc,
                "qc2c": qc2c,
                "onesr": np.ones((1, 256), np.float16),
                "idf": np.eye(128, dtype=np.float32),
                "idb": np.eye(128, dtype=np.float32).astype(bf),
            }
        )
    return in_maps, T_pad


def kernel(x, batch, cos_coef, q_star, W_ih, W_hh, b_ih, b_hh):
    global LAST_RESULT
    in_maps, T_pad = make_in_maps(
        x, batch, cos_coef, q_star, W_ih, W_hh, b_ih, b_hh
    )
    nc = _get_program(T_pad)
    res = run_bass_kernel_spmd(nc, in_maps, list(range(CORES)), trace=TRACE)
    LAST_RESULT = res
    out = np.zeros((B, 2 * F), dtype=np.float32)
    for c in range(CORES):
        out[c * SEG_PER_CORE : (c + 1) * SEG_PER_CORE] = res.results[c]["qout"]
    return out


# revision 19
# speedup vs baseline: 1.3814x; 1.1758x over previous
"""Trainium2 Bass kernel for CognitionNetwork (GNN message passing + LSTM attention).

Contract: kernel(**inputs) takes FULL inputs, returns FULL [2048, 400] q_star.
Shards 2048 conversations contiguously across 8 NeuronCores (256 segments each);
each block of 32 segments owns T_pad 128-node tiles (host re-layout).

v2 design (vs v0 per-tile gather):
  - attention scores e come from block-level matmuls contracting FEATURES:
    weights = per-block Q^T (reused across the block's tiles), rhs = a
    feature-major fp16 copy of x. The segment mask is folded into 33 extra
    "features" (indicator rows * 100 on both sides, ones row * -100), so
    e_aug = e + 100*onehot - 100 and exp(e_aug) is already the masked,
    unnormalized attention weight (off-segment entries underflow to 0).
  - exp runs on the scalar engine straight out of PSUM into a bf16 tile;
    per-tile PE transposes flip it node-major; the r matmul streams a bf16
    node-major x copy (ones column appended -> denominator for free).
  - all matmul operands are 16-bit (fp16 for e/LSTM, bf16 for r/phase0):
    1 cycle/row at any output width; fp32 masters kept for h/c/r state.
"""

import os
from contextlib import ExitStack

import ml_dtypes
import numpy as np

import concourse.bass as bass
import concourse.bacc as bacc
import concourse.tile as tile
from concourse import mybir
from concourse.bass_utils import run_bass_kernel_spmd

CORES = 8
B = 2048
F = 200
FW = 201              # node-major x tile width: 200 feats + ones col
SEG_PER_CORE = B // CORES   # 256
BS = 32               # segments per block
BLOCKS = SEG_PER_CORE // BS  # 8
STEPS = 3
KAUG = F + BS + 1     # 233 feature rows incl mask aug
K2 = KAUG - 128       # 105 rows in chunk 2

TRACE = bool(int(os.environ.get("KERNEL_TRACE", "0")))
LAST_RESULT = None
_PROG_CACHE = {}


def _build_program(T_pad: int, nsteps: int = STEPS) -> bass.Bass:
    NT = BLOCKS * T_pad          # node tiles per core
    XFW = NT * 128               # feature-major x width (nodes)
    BW = T_pad * 128             # nodes per block

    nc = bacc.Bacc("TRN2", target_bir_lowering=False, debug=False)
    f32 = mybir.dt.float32
    f32r = mybir.dt.float32r
    f16 = mybir.dt.float16
    bf16 = mybir.dt.bfloat16
    AF = mybir.ActivationFunctionType

    xf1_d = nc.dram_tensor("xf1", [128, XFW], f16, kind="ExternalInput").ap()
    xf2_d = nc.dram_tensor("xf2", [K2, XFW], f16, kind="ExternalInput").ap()
    cwt_d = nc.dram_tensor("cwt", [128, NT * BS], f16, kind="ExternalInput").ap()
    xp_d = nc.dram_tensor("xp", [128, NT * FW], f16, kind="ExternalInput").ap()
    qs0t_d = nc.dram_tensor("qs0t", [401, 256], f16, kind="ExternalInput").ap()
    w0_d = nc.dram_tensor("w0", [634, 800], f16, kind="ExternalInput").ap()
    wc_d = nc.dram_tensor("wc", [434, 800], f16, kind="ExternalInput").ap()
    qc2c_d = nc.dram_tensor("qc2c", [BS + 1, 256], f16, kind="ExternalInput").ap()
    ones_d = nc.dram_tensor("onesr", [1, 256], f16, kind="ExternalInput").ap()
    idf_d = nc.dram_tensor("idf", [128, 128], f32r, kind="ExternalInput").ap()
    idb_d = nc.dram_tensor("idb", [128, 128], bf16, kind="ExternalInput").ap()
    qout_d = nc.dram_tensor("qout", [256, 400], f32, kind="ExternalOutput").ap()

    with tile.TileContext(nc) as tc:
        with ExitStack() as ctx:
            res = ctx.enter_context(tc.tile_pool(name="res", bufs=1))
            state = ctx.enter_context(tc.tile_pool(name="state", bufs=1))
            eap = ctx.enter_context(tc.tile_pool(name="eap", bufs=2))
            xpp = ctx.enter_context(tc.tile_pool(name="xpp", bufs=3))
            eanp = ctx.enter_context(tc.tile_pool(name="eanp", bufs=2))
            sbt = ctx.enter_context(tc.tile_pool(name="sbt", bufs=2))
            psE = ctx.enter_context(tc.tile_pool(name="psE", bufs=2, space="PSUM"))
            psG = ctx.enter_context(tc.tile_pool(name="psG", bufs=2, space="PSUM"))
            psT = ctx.enter_context(tc.tile_pool(name="psT", bufs=2, space="PSUM"))
            psR = ctx.enter_context(tc.tile_pool(name="psR", bufs=2, space="PSUM"))

            # ---------------- resident loads ----------------
            idf = res.tile([128, 128], f32r)
            nc.sync.dma_start(idf[:], idf_d[:])
            idb = res.tile([128, 128], bf16)
            nc.sync.dma_start(idb[:], idb_d[:])

            cwt_sb = res.tile([128, NT * BS], f16)
            xnm_sb = res.tile([128, NT * FW], bf16)
            xf1_sb = res.tile([128, XFW], f16)
            xf2_sb = res.tile([K2, XFW], f16)

            # transposed-input chunks: Q1/Q2 (h^T + mask const), R1/R2 (r^T + ones)
            Q1 = res.tile([128, 256], f16, tag="Q1", name="Q1")
            Q2 = res.tile([K2, 256], f16, tag="Q2", name="Q2")
            nc.sync.dma_start(Q2[72:K2, :], qc2c_d[:])
            R1 = res.tile([128, 256], f16, tag="R1", name="R1")
            R2 = res.tile([73, 256], f16, tag="R2", name="R2")
            nc.sync.dma_start(R2[72:73, :], ones_d[:])

            # fp32 state masters (seg-major, two 128-partition halves)
            h_sb = [state.tile([128, F], f32r, tag=f"h{i}", name=f"h{i}") for i in range(2)]
            c_sb = [state.tile([128, F], f32, tag=f"c{i}", name=f"c{i}") for i in range(2)]
            r_sb = [state.tile([128, F], f32r, tag=f"r{i}", name=f"r{i}") for i in range(2)]
            for i in range(2):
                nc.vector.memset(c_sb[i][:], 0.0)

            # ---------------- phase 0: h0 = segment_sum(cos * x) ----------------
            # quad-stacked; streams fp16 x (with ones col) per block, casting it
            # into the resident bf16 node-major copy as it goes
            for q in range(2):
                h0ps = psR.tile([128, F], f32, tag="rblk")
                for a in range(4):
                    g = 4 * q + a
                    nc.sync.dma_start(
                        cwt_sb[:, g * T_pad * BS : (g + 1) * T_pad * BS],
                        cwt_d[:, g * T_pad * BS : (g + 1) * T_pad * BS],
                    )
                    xpt = xpp.tile([128, T_pad * FW], f16, tag="xp")
                    XH = (T_pad * FW) // 2
                    nc.sync.dma_start(xpt[:, 0:XH], xp_d[:, g * T_pad * FW : g * T_pad * FW + XH])
                    nc.sync.dma_start(
                        xpt[:, XH : T_pad * FW],
                        xp_d[:, g * T_pad * FW + XH : (g + 1) * T_pad * FW],
                    )
                    for i in range(T_pad):
                        t = g * T_pad + i
                        nc.tensor.matmul(
                            h0ps[32 * a : 32 * a + 32, :],
                            lhsT=cwt_sb[:, t * BS : (t + 1) * BS],
                            rhs=xpt[:, i * FW : i * FW + F],
                            start=(i == 0),
                            stop=(i == T_pad - 1),
                            tile_position=(0, 32 * a),
                        )
                    nc.vector.tensor_copy(
                        xnm_sb[:, g * T_pad * FW : (g + 1) * T_pad * FW], xpt[:]
                    )
                nc.vector.tensor_copy(h_sb[q][:], h0ps[:])

            # remaining loads, in consumption order: LSTM0 weights, then
            # feature-major x for attention, then step>=1 weights
            wE = []
            for k, o in zip([128, 128, 128, 17, 128, K2], [0, 128, 256, 384, 401, 529]):
                t = res.tile([k, 800], f16, tag=f"wE{o}", name=f"wE{o}")
                nc.sync.dma_start(t[:], w0_d[o : o + k, :])
                wE.append(t)
            qsE = []
            for k, o in zip([128, 128, 128, 17], [0, 128, 256, 384]):
                t = res.tile([k, 256], f16, tag=f"qsE{o}", name=f"qsE{o}")
                nc.sync.dma_start(t[:], qs0t_d[o : o + k, :])
                qsE.append(t)
            HB = BW // 2
            for g in range(BLOCKS):
                for hh in range(2):
                    c0 = g * BW + hh * HB
                    c1 = g * BW + (HB if hh == 0 else BW)
                    nc.sync.dma_start(xf1_sb[:, c0:c1], xf1_d[:, c0:c1])
                    nc.sync.dma_start(xf2_sb[:, c0:c1], xf2_d[:, c0:c1])
            wD = []
            for k, o in zip([128, K2, 128, 73], [0, 128, 233, 361]):
                t = res.tile([k, 800], f16, tag=f"wD{o}", name=f"wD{o}")
                nc.sync.dma_start(t[:], wc_d[o : o + k, :])
                wD.append(t)

            def emit_hT(src_halves, dst1, dst2, halves=(0, 1)):
                """transpose seg-major [128,200] f32r halves into fp16 feat-major
                chunks: dst1[:, co:co+128] rows 0..127, dst2[0:72, ...] rows 128..199."""
                for half in halves:
                    src = src_halves[half]
                    co = 128 * half
                    t1 = psT.tile([128, 128], f32r, tag="tp")
                    nc.tensor.transpose(t1[:], src[:, 0:128], idf[:])
                    nc.vector.tensor_copy(dst1[:, co : co + 128], t1[:].bitcast(f32))
                    t2 = psT.tile([72, 128], f32r, tag="tp")
                    nc.tensor.transpose(t2[:], src[:, 128:200], idf[:])
                    nc.vector.tensor_copy(dst2[0:72, co : co + 128], t2[:].bitcast(f32))

            emit_hT(h_sb, Q1, Q2)

            # ---------------- steps ----------------
            NCH = (BW + 511) // 512  # 512-col e-matmul chunks per block

            def emit_e(q):
                """e_aug matmuls + exp for 4 stacked blocks -> EA [128, BW] bf16."""
                ea = eap.tile([128, BW], bf16, tag="ea", name=f"ea")
                for k in range(NCH):
                    c0 = k * 512
                    cw = min(512, BW - c0)
                    pe = psE.tile([128, 512], f32, tag="pe")
                    for a in range(4):
                        g = 4 * q + a
                        nc.tensor.matmul(
                            pe[32 * a : 32 * a + 32, 0:cw],
                            lhsT=Q1[:, BS * g : BS * (g + 1)],
                            rhs=xf1_sb[:, g * BW + c0 : g * BW + c0 + cw],
                            start=True,
                            stop=False,
                            tile_position=(0, 32 * a),
                        )
                        nc.tensor.matmul(
                            pe[32 * a : 32 * a + 32, 0:cw],
                            lhsT=Q2[0:K2, BS * g : BS * (g + 1)],
                            rhs=xf2_sb[0:K2, g * BW + c0 : g * BW + c0 + cw],
                            start=False,
                            stop=True,
                            tile_position=(0, 32 * a),
                        )
                    nc.scalar.activation(ea[:, c0 : c0 + cw], pe[:, 0:cw], AF.Exp)
                return ea

            def emit_attn_tail(q, ea):
                """transpose EA node-major (4 tiles/instr), r matmuls, normalize."""
                rps = psR.tile([128, F + 1], f32, tag="rblk")
                NG = (T_pad + 3) // 4
                prev = None
                for k in range(NG):
                    n4 = min(4, T_pad - 4 * k)
                    tp = psT.tile([128, 512], bf16, tag="tp")
                    for i4 in range(n4):
                        i = 4 * k + i4
                        nc.tensor.transpose(
                            tp[:, 128 * i4 : 128 * i4 + 128],
                            ea[:, 128 * i : 128 * i + 128],
                            idb[:],
                        )
                    ean = eanp.tile([128, 512], bf16, tag="ean")
                    nc.vector.tensor_copy(ean[:, 0 : 128 * n4], tp[:, 0 : 128 * n4])
                    if prev is not None:
                        for i4 in range(prev[1]):
                            _emit_r(q, 4 * prev[0] + i4, prev[2], rps, i4)
                    prev = (k, n4, ean)
                for i4 in range(prev[1]):
                    _emit_r(q, 4 * prev[0] + i4, prev[2], rps, i4)
                dinv = sbt.tile([128, 1], f32, tag="dinv")
                nc.vector.reciprocal(dinv[:], rps[:, F : F + 1])
                nc.vector.tensor_scalar_mul(r_sb[q][:], rps[:, 0:F], dinv[:])

            def _emit_r(q, i, ean, rps, i4):
                for a in range(4):
                    t = (4 * q + a) * T_pad + i
                    nc.tensor.matmul(
                        rps[32 * a : 32 * a + 32, :],
                        lhsT=ean[:, 128 * i4 + 32 * a : 128 * i4 + 32 * a + 32],
                        rhs=xnm_sb[:, t * FW : t * FW + F + 1],
                        start=(i == 0),
                        stop=(i == T_pad - 1),
                        tile_position=(0, 32 * a),
                    )

            for s in range(nsteps):
                # ---- LSTM cell (seg-major halves) ----
                if s == 0:
                    chunks = list(zip(qsE, [128, 128, 128, 17])) + [(Q1, 128), (Q2, K2)]
                    wts = wE
                else:
                    chunks = [(Q1, 128), (Q2, K2), (R1, 128), (R2, 73)]
                    wts = wD
                def lstm_half(half):
                    co = 128 * half
                    acts = {}
                    for part in range(2):
                        ps = psG.tile([128, 400], f32, tag="gates")
                        nch = len(chunks)
                        for ci, (ctile, kdim) in enumerate(chunks):
                            nc.tensor.matmul(
                                ps[:],
                                lhsT=ctile[0:kdim, co : co + 128],
                                rhs=wts[ci][0:kdim, 400 * part : 400 * part + 400],
                                start=(ci == 0),
                                stop=(ci == nch - 1),
                            )
                        if part == 0:
                            si = sbt.tile([128, F], f32, tag="si")
                            nc.scalar.activation(si[:], ps[:, 0:F], AF.Sigmoid)
                            sf = sbt.tile([128, F], f32, tag="sf")
                            nc.scalar.activation(sf[:], ps[:, F:400], AF.Sigmoid)
                            acts["i"], acts["f"] = si, sf
                        else:
                            tg = sbt.tile([128, F], f32, tag="tg")
                            nc.scalar.activation(tg[:], ps[:, 0:F], AF.Tanh)
                            so = sbt.tile([128, F], f32, tag="so")
                            nc.scalar.activation(so[:], ps[:, F:400], AF.Sigmoid)
                            acts["g"], acts["o"] = tg, so
                    ch = c_sb[half]
                    tmp = sbt.tile([128, F], f32, tag="tmp")
                    nc.vector.tensor_mul(tmp[:], acts["f"][:], ch[:])
                    nc.vector.tensor_mul(ch[:], acts["i"][:], acts["g"][:])
                    nc.vector.tensor_add(ch[:], tmp[:], ch[:])
                    tct = sbt.tile([128, F], f32, tag="tct")
                    nc.scalar.activation(tct[:], ch[:], AF.Tanh)
                    nc.vector.tensor_mul(h_sb[half][:], acts["o"][:], tct[:])

                lstm_half(0)
                lstm_half(1)
                if s == nsteps - 1:
                    for half in range(2):
                        nc.sync.dma_start(
                            qout_d[128 * half : 128 * half + 128, 0:F],
                            h_sb[half][:].bitcast(f32),
                        )

                # ---- per-half h^T then e-matmuls: attention starts while the
                # other half's LSTM tail still runs on scalar/vector ----
                emit_hT(h_sb, Q1, Q2, halves=(0,))
                ea0 = emit_e(0)
                emit_hT(h_sb, Q1, Q2, halves=(1,))
                ea1 = emit_e(1)
                emit_attn_tail(0, ea0)
                if s == nsteps - 1:
                    nc.sync.dma_start(qout_d[0:128, F : 2 * F], r_sb[0][:].bitcast(f32))
                emit_attn_tail(1, ea1)
                if s == nsteps - 1:
                    nc.sync.dma_start(qout_d[128:256, F : 2 * F], r_sb[1][:].bitcast(f32))
                if s < nsteps - 1:
                    emit_hT(r_sb, R1, R2)

            if nsteps == 0:
                for half in range(2):
                    nc.sync.dma_start(
                        qout_d[128 * half : 128 * half + 128, 0:F], h_sb[half][:].bitcast(f32)
                    )

    nc.compile()
    return nc


def _get_program(T_pad: int) -> bass.Bass:
    nsteps = int(os.environ.get("KERNEL_NSTEPS", str(STEPS)))
    key = (T_pad, nsteps)
    if key not in _PROG_CACHE:
        _PROG_CACHE[key] = _build_program(T_pad, nsteps)
    return _PROG_CACHE[key]


def make_in_maps(x, batch, cos_coef, q_star, W_ih, W_hh, b_ih, b_hh):
    """Host-side shard + re-layout. Returns (in_maps, T_pad)."""
    x = np.ascontiguousarray(np.asarray(x, dtype=np.float32))
    batch = np.asarray(batch).astype(np.int64)
    cos = np.asarray(cos_coef, dtype=np.float32)
    qs = np.asarray(q_star, dtype=np.float32)
    W_ih = np.asarray(W_ih, dtype=np.float32)
    W_hh = np.asarray(W_hh, dtype=np.float32)
    bsum = (np.asarray(b_ih, dtype=np.float32) + np.asarray(b_hh, dtype=np.float32))

    counts = np.bincount(batch, minlength=B)
    starts = np.zeros(B + 1, dtype=np.int64)
    starts[1:] = np.cumsum(counts)
    blk_counts = counts.reshape(-1, BS).sum(axis=1)
    T_pad = int(max(1, -(-blk_counts.max() // 128)))
    NT = BLOCKS * T_pad
    BW = T_pad * 128

    bf = ml_dtypes.bfloat16

    # LSTM weight stacks (fp16)
    W_ihT = W_ih.T  # [400, 800]
    W_hhT = W_hh.T  # [200, 800]
    w0 = np.concatenate(
        [W_ihT, bsum[None, :], W_hhT, np.zeros((BS + 1, 800), np.float32)], axis=0
    ).astype(np.float16)  # [634, 800]; rows 529.. = W_hhT[128:200] + aug zeros
    WcT = W_ihT[:F] + W_hhT          # [200, 800]
    WrT = W_ihT[F:]                  # [200, 800]
    wc = np.concatenate(
        [WcT[0:128], WcT[128:200], np.zeros((BS + 1, 800), np.float32),
         WrT[0:128], WrT[128:200], bsum[None, :]], axis=0
    ).astype(np.float16)             # [434, 800]

    qc2c = np.zeros((BS + 1, 256), np.float16)
    qc2c[0:BS] = np.tile(100.0 * np.eye(BS, dtype=np.float32), (1, BLOCKS))
    qc2c[BS] = -100.0

    in_maps = []
    for c in range(CORES):
        seg0 = c * SEG_PER_CORE
        xf = np.zeros((KAUG, NT * 128), dtype=np.float16)
        cwt = np.zeros((128, NT * BS), dtype=np.float16)
        xp = np.zeros((128, NT * FW), dtype=np.float16)
        for g in range(BLOCKS):
            sa = seg0 + g * BS
            n0, n1 = int(starts[sa]), int(starts[sa + BS])
            cnt = n1 - n0
            js = (batch[n0:n1] - sa).astype(np.int64)

            xb = np.zeros((BW, FW), dtype=np.float32)
            xb[:cnt, :F] = x[n0:n1]
            xb[:cnt, F] = 1.0
            xp[:, g * T_pad * FW : (g + 1) * T_pad * FW] = (
                xb.reshape(T_pad, 128, FW).transpose(1, 0, 2).reshape(128, T_pad * FW)
            ).astype(np.float16)

            xfb = np.zeros((KAUG, BW), dtype=np.float32)
            xfb[0:F, :cnt] = x[n0:n1].T
            xfb[F + js, np.arange(cnt)] = 1.0
            xfb[F + BS, :] = 1.0
            xf[:, g * BW : (g + 1) * BW] = xfb.astype(np.float16)

            wb = np.zeros((BW, BS), dtype=np.float32)
            wb[np.arange(cnt), js] = cos[n0:n1]
            cwt[:, g * T_pad * BS : (g + 1) * T_pad * BS] = (
                wb.reshape(T_pad, 128, BS).transpose(1, 0, 2).reshape(128, T_pad * BS)
            ).astype(np.float16)

        qs0t = np.ones((401, 256), dtype=np.float16)
        qs0t[0:400] = qs[seg0 : seg0 + SEG_PER_CORE].T.astype(np.float16)
        in_maps.append(
            {
                "xf1": np.ascontiguousarray(xf[0:128]),
                "xf2": np.ascontiguousarray(xf[128:KAUG]),
                "cwt": cwt,
                "xp": xp,
                "qs0t": qs0t,
                "w0": w0,
                "wc": wc,
                "qc2c": qc2c,
                "onesr": np.ones((1, 256), np.float16),
                "idf": np.eye(128, dtype=np.float32),
                "idb": np.eye(128, dtype=np.float32).astype(bf),
            }
        )
    return in_maps, T_pad


def kernel(x, batch, cos_coef, q_star, W_ih, W_hh, b_ih, b_hh):
    global LAST_RESULT
    in_maps, T_pad = make_in_maps(
        x, batch, cos_coef, q_star, W_ih, W_hh, b_ih, b_hh
    )
    nc = _get_program(T_pad)
    res = run_bass_kernel_spmd(nc, in_maps, list(range(CORES)), trace=TRACE)
    LAST_RESULT = res
    out = np.zeros((B, 2 * F), dtype=np.float32)
    for c in range(CORES):
        out[c * SEG_PER_CORE : (c + 1) * SEG_PER_CORE] = res.results[c]["qout"]
    return out


# revision 20
# speedup vs baseline: 1.3977x; 1.0118x over previous
"""Trainium2 Bass kernel for CognitionNetwork (GNN message passing + LSTM attention).

Contract: kernel(**inputs) takes FULL inputs, returns FULL [2048, 400] q_star.
Shards 2048 conversations contiguously across 8 NeuronCores (256 segments each);
each block of 32 segments owns T_pad 128-node tiles (host re-layout).

v2 design (vs v0 per-tile gather):
  - attention scores e come from block-level matmuls contracting FEATURES:
    weights = per-block Q^T (reused across the block's tiles), rhs = a
    feature-major fp16 copy of x. The segment mask is folded into 33 extra
    "features" (indicator rows * 100 on both sides, ones row * -100), so
    e_aug = e + 100*onehot - 100 and exp(e_aug) is already the masked,
    unnormalized attention weight (off-segment entries underflow to 0).
  - exp runs on the scalar engine straight out of PSUM into a bf16 tile;
    per-tile PE transposes flip it node-major; the r matmul streams a bf16
    node-major x copy (ones column appended -> denominator for free).
  - all matmul operands are 16-bit (fp16 for e/LSTM, bf16 for r/phase0):
    1 cycle/row at any output width; fp32 masters kept for h/c/r state.
"""

import os
from contextlib import ExitStack

import ml_dtypes
import numpy as np

import concourse.bass as bass
import concourse.bacc as bacc
import concourse.tile as tile
from concourse import mybir
from concourse.bass_utils import run_bass_kernel_spmd

CORES = 8
B = 2048
F = 200
FW = 201              # node-major x tile width: 200 feats + ones col
SEG_PER_CORE = B // CORES   # 256
BS = 32               # segments per block
BLOCKS = SEG_PER_CORE // BS  # 8
STEPS = 3
KAUG = F + BS + 1     # 233 feature rows incl mask aug
K2 = KAUG - 128       # 105 rows in chunk 2

TRACE = bool(int(os.environ.get("KERNEL_TRACE", "0")))
LAST_RESULT = None
_PROG_CACHE = {}


def _build_program(T_pad: int, nsteps: int = STEPS) -> bass.Bass:
    NT = BLOCKS * T_pad          # node tiles per core
    XFW = NT * 128               # feature-major x width (nodes)
    BW = T_pad * 128             # nodes per block

    nc = bacc.Bacc("TRN2", target_bir_lowering=False, debug=False)
    f32 = mybir.dt.float32
    f32r = mybir.dt.float32r
    f16 = mybir.dt.float16
    bf16 = mybir.dt.bfloat16
    AF = mybir.ActivationFunctionType

    xf1_d = nc.dram_tensor("xf1", [128, XFW], f16, kind="ExternalInput").ap()
    xf2_d = nc.dram_tensor("xf2", [K2, XFW], f16, kind="ExternalInput").ap()
    cwt_d = nc.dram_tensor("cwt", [128, NT * BS], f16, kind="ExternalInput").ap()
    xp_d = nc.dram_tensor("xp", [128, NT * FW], f16, kind="ExternalInput").ap()
    qs0t_d = nc.dram_tensor("qs0t", [401, 256], f16, kind="ExternalInput").ap()
    w0_d = nc.dram_tensor("w0", [634, 800], f16, kind="ExternalInput").ap()
    wc_d = nc.dram_tensor("wc", [434, 800], f16, kind="ExternalInput").ap()
    qc2c_d = nc.dram_tensor("qc2c", [BS + 1, 256], f16, kind="ExternalInput").ap()
    ones_d = nc.dram_tensor("onesr", [1, 256], f16, kind="ExternalInput").ap()
    idf_d = nc.dram_tensor("idf", [128, 128], f32r, kind="ExternalInput").ap()
    idb_d = nc.dram_tensor("idb", [128, 128], bf16, kind="ExternalInput").ap()
    qout_d = nc.dram_tensor("qout", [256, 400], f32, kind="ExternalOutput").ap()

    with tile.TileContext(nc) as tc:
        with ExitStack() as ctx:
            res = ctx.enter_context(tc.tile_pool(name="res", bufs=1))
            state = ctx.enter_context(tc.tile_pool(name="state", bufs=1))
            eap = ctx.enter_context(tc.tile_pool(name="eap", bufs=2))
            xpp = ctx.enter_context(tc.tile_pool(name="xpp", bufs=3))
            eanp = ctx.enter_context(tc.tile_pool(name="eanp", bufs=3))
            sbt = ctx.enter_context(tc.tile_pool(name="sbt", bufs=2))
            psE = ctx.enter_context(tc.tile_pool(name="psE", bufs=2, space="PSUM"))
            psG = ctx.enter_context(tc.tile_pool(name="psG", bufs=3, space="PSUM"))
            psT = ctx.enter_context(tc.tile_pool(name="psT", bufs=2, space="PSUM"))
            psR = ctx.enter_context(tc.tile_pool(name="psR", bufs=1, space="PSUM"))

            # ---------------- resident loads ----------------
            idf = res.tile([128, 128], f32r)
            nc.sync.dma_start(idf[:], idf_d[:])
            idb = res.tile([128, 128], bf16)
            nc.sync.dma_start(idb[:], idb_d[:])

            cwt_sb = res.tile([128, NT * BS], f16)
            xnm_sb = res.tile([128, NT * FW], bf16)
            xf1_sb = res.tile([128, XFW], f16)
            xf2_sb = res.tile([K2, XFW], f16)

            # transposed-input chunks: Q1/Q2 (h^T + mask const), R1/R2 (r^T + ones)
            Q1 = res.tile([128, 256], f16, tag="Q1", name="Q1")
            Q2 = res.tile([K2, 256], f16, tag="Q2", name="Q2")
            nc.sync.dma_start(Q2[72:K2, :], qc2c_d[:])
            R1 = res.tile([128, 256], f16, tag="R1", name="R1")
            R2 = res.tile([73, 256], f16, tag="R2", name="R2")
            nc.sync.dma_start(R2[72:73, :], ones_d[:])

            # fp32 state masters (seg-major, two 128-partition halves)
            h_sb = [state.tile([128, F], f32r, tag=f"h{i}", name=f"h{i}") for i in range(2)]
            c_sb = [state.tile([128, F], f32, tag=f"c{i}", name=f"c{i}") for i in range(2)]
            r_sb = [state.tile([128, F], f32r, tag=f"r{i}", name=f"r{i}") for i in range(2)]
            for i in range(2):
                nc.vector.memset(c_sb[i][:], 0.0)

            # ---------------- phase 0: h0 = segment_sum(cos * x) ----------------
            # quad-stacked; streams fp16 x (with ones col) per block, casting it
            # into the resident bf16 node-major copy as it goes
            for q in range(2):
                h0ps = psR.tile([128, F], f32, tag="rblk")
                for a in range(4):
                    g = 4 * q + a
                    nc.sync.dma_start(
                        cwt_sb[:, g * T_pad * BS : (g + 1) * T_pad * BS],
                        cwt_d[:, g * T_pad * BS : (g + 1) * T_pad * BS],
                    )
                    xpt = xpp.tile([128, T_pad * FW], f16, tag="xp")
                    XH = (T_pad * FW) // 2
                    nc.sync.dma_start(xpt[:, 0:XH], xp_d[:, g * T_pad * FW : g * T_pad * FW + XH])
                    nc.sync.dma_start(
                        xpt[:, XH : T_pad * FW],
                        xp_d[:, g * T_pad * FW + XH : (g + 1) * T_pad * FW],
                    )
                    for i in range(T_pad):
                        t = g * T_pad + i
                        nc.tensor.matmul(
                            h0ps[32 * a : 32 * a + 32, :],
                            lhsT=cwt_sb[:, t * BS : (t + 1) * BS],
                            rhs=xpt[:, i * FW : i * FW + F],
                            start=(i == 0),
                            stop=(i == T_pad - 1),
                            tile_position=(0, 32 * a),
                        )
                    nc.vector.tensor_copy(
                        xnm_sb[:, g * T_pad * FW : (g + 1) * T_pad * FW], xpt[:]
                    )
                nc.vector.tensor_copy(h_sb[q][:], h0ps[:])

            # remaining loads, in consumption order: LSTM0 weights, then
            # feature-major x for attention, then step>=1 weights
            wE = []
            for k, o in zip([128, 128, 128, 17, 128, K2], [0, 128, 256, 384, 401, 529]):
                t = res.tile([k, 800], f16, tag=f"wE{o}", name=f"wE{o}")
                nc.sync.dma_start(t[:], w0_d[o : o + k, :])
                wE.append(t)
            qsE = []
            for k, o in zip([128, 128, 128, 17], [0, 128, 256, 384]):
                t = res.tile([k, 256], f16, tag=f"qsE{o}", name=f"qsE{o}")
                nc.sync.dma_start(t[:], qs0t_d[o : o + k, :])
                qsE.append(t)
            HB = BW // 2
            for g in range(BLOCKS):
                for hh in range(2):
                    c0 = g * BW + hh * HB
                    c1 = g * BW + (HB if hh == 0 else BW)
                    nc.sync.dma_start(xf1_sb[:, c0:c1], xf1_d[:, c0:c1])
                    nc.sync.dma_start(xf2_sb[:, c0:c1], xf2_d[:, c0:c1])
            wD = []
            for k, o in zip([128, K2, 128, 73], [0, 128, 233, 361]):
                t = res.tile([k, 800], f16, tag=f"wD{o}", name=f"wD{o}")
                nc.sync.dma_start(t[:], wc_d[o : o + k, :])
                wD.append(t)

            def emit_hT(src_halves, dst1, dst2, halves=(0, 1)):
                """transpose seg-major [128,200] f32r halves into fp16 feat-major
                chunks: dst1[:, co:co+128] rows 0..127, dst2[0:72, ...] rows 128..199."""
                for half in halves:
                    src = src_halves[half]
                    co = 128 * half
                    t1 = psT.tile([128, 128], f32r, tag="tp")
                    nc.tensor.transpose(t1[:], src[:, 0:128], idf[:])
                    nc.vector.tensor_copy(dst1[:, co : co + 128], t1[:].bitcast(f32))
                    t2 = psT.tile([72, 128], f32r, tag="tp")
                    nc.tensor.transpose(t2[:], src[:, 128:200], idf[:])
                    nc.vector.tensor_copy(dst2[0:72, co : co + 128], t2[:].bitcast(f32))

            emit_hT(h_sb, Q1, Q2)

            # ---------------- steps ----------------
            NCH = (BW + 511) // 512  # 512-col e-matmul chunks per block

            def emit_e(q):
                """e_aug matmuls + exp for 4 stacked blocks -> EA [128, BW] bf16."""
                ea = eap.tile([128, BW], bf16, tag="ea", name=f"ea")
                for k in range(NCH):
                    c0 = k * 512
                    cw = min(512, BW - c0)
                    pe = psE.tile([128, 512], f32, tag="pe")
                    for a in range(4):
                        g = 4 * q + a
                        nc.tensor.matmul(
                            pe[32 * a : 32 * a + 32, 0:cw],
                            lhsT=Q1[:, BS * g : BS * (g + 1)],
                            rhs=xf1_sb[:, g * BW + c0 : g * BW + c0 + cw],
                            start=True,
                            stop=False,
                            tile_position=(0, 32 * a),
                        )
                        nc.tensor.matmul(
                            pe[32 * a : 32 * a + 32, 0:cw],
                            lhsT=Q2[0:K2, BS * g : BS * (g + 1)],
                            rhs=xf2_sb[0:K2, g * BW + c0 : g * BW + c0 + cw],
                            start=False,
                            stop=True,
                            tile_position=(0, 32 * a),
                        )
                    nc.scalar.activation(ea[:, c0 : c0 + cw], pe[:, 0:cw], AF.Exp)
                return ea

            def emit_attn_tail(q, ea):
                """transpose EA node-major (4 tiles/instr), r matmuls, normalize."""
                rps = psR.tile([128, F + 1], f32, tag="rblk")
                NG = (T_pad + 3) // 4
                prev = None
                for k in range(NG):
                    n4 = min(4, T_pad - 4 * k)
                    tp = psT.tile([128, 512], bf16, tag="tp")
                    for i4 in range(n4):
                        i = 4 * k + i4
                        nc.tensor.transpose(
                            tp[:, 128 * i4 : 128 * i4 + 128],
                            ea[:, 128 * i : 128 * i + 128],
                            idb[:],
                        )
                    ean = eanp.tile([128, 512], bf16, tag="ean")
                    nc.vector.tensor_copy(ean[:, 0 : 128 * n4], tp[:, 0 : 128 * n4])
                    if prev is not None:
                        for i4 in range(prev[1]):
                            _emit_r(q, 4 * prev[0] + i4, prev[2], rps, i4)
                    prev = (k, n4, ean)
                for i4 in range(prev[1]):
                    _emit_r(q, 4 * prev[0] + i4, prev[2], rps, i4)
                dinv = sbt.tile([128, 1], f32, tag="dinv")
                nc.vector.reciprocal(dinv[:], rps[:, F : F + 1])
                nc.scalar.activation(r_sb[q][:], rps[:, 0:F], AF.Copy, scale=dinv[:])

            def _emit_r(q, i, ean, rps, i4):
                for a in range(4):
                    t = (4 * q + a) * T_pad + i
                    nc.tensor.matmul(
                        rps[32 * a : 32 * a + 32, :],
                        lhsT=ean[:, 128 * i4 + 32 * a : 128 * i4 + 32 * a + 32],
                        rhs=xnm_sb[:, t * FW : t * FW + F + 1],
                        start=(i == 0),
                        stop=(i == T_pad - 1),
                        tile_position=(0, 32 * a),
                    )

            for s in range(nsteps):
                # ---- LSTM cell (seg-major halves) ----
                if s == 0:
                    chunks = list(zip(qsE, [128, 128, 128, 17])) + [(Q1, 128), (Q2, K2)]
                    wts = wE
                else:
                    chunks = [(Q1, 128), (Q2, K2), (R1, 128), (R2, 73)]
                    wts = wD
                def lstm_half(half):
                    co = 128 * half
                    acts = {}
                    for part in range(2):
                        ps = psG.tile([128, 400], f32, tag="gates")
                        nch = len(chunks)
                        for ci, (ctile, kdim) in enumerate(chunks):
                            nc.tensor.matmul(
                                ps[:],
                                lhsT=ctile[0:kdim, co : co + 128],
                                rhs=wts[ci][0:kdim, 400 * part : 400 * part + 400],
                                start=(ci == 0),
                                stop=(ci == nch - 1),
                            )
                        if part == 0:
                            si = sbt.tile([128, F], f32, tag="si")
                            nc.scalar.activation(si[:], ps[:, 0:F], AF.Sigmoid)
                            sf = sbt.tile([128, F], f32, tag="sf")
                            nc.scalar.activation(sf[:], ps[:, F:400], AF.Sigmoid)
                            acts["i"], acts["f"] = si, sf
                        else:
                            tg = sbt.tile([128, F], f32, tag="tg")
                            nc.scalar.activation(tg[:], ps[:, 0:F], AF.Tanh)
                            so = sbt.tile([128, F], f32, tag="so")
                            nc.scalar.activation(so[:], ps[:, F:400], AF.Sigmoid)
                            acts["g"], acts["o"] = tg, so
                    ch = c_sb[half]
                    tmp = sbt.tile([128, F], f32, tag="tmp")
                    nc.vector.tensor_mul(tmp[:], acts["f"][:], ch[:])
                    nc.vector.tensor_mul(ch[:], acts["i"][:], acts["g"][:])
                    nc.vector.tensor_add(ch[:], tmp[:], ch[:])
                    tct = sbt.tile([128, F], f32, tag="tct")
                    nc.scalar.activation(tct[:], ch[:], AF.Tanh)
                    nc.vector.tensor_mul(h_sb[half][:], acts["o"][:], tct[:])

                lstm_half(0)
                lstm_half(1)
                if s == nsteps - 1:
                    for half in range(2):
                        nc.sync.dma_start(
                            qout_d[128 * half : 128 * half + 128, 0:F],
                            h_sb[half][:].bitcast(f32),
                        )

                # ---- per-half h^T then e-matmuls: attention starts while the
                # other half's LSTM tail still runs on scalar/vector ----
                emit_hT(h_sb, Q1, Q2, halves=(0,))
                ea0 = emit_e(0)
                emit_hT(h_sb, Q1, Q2, halves=(1,))
                ea1 = emit_e(1)
                emit_attn_tail(0, ea0)
                if s == nsteps - 1:
                    nc.sync.dma_start(qout_d[0:128, F : 2 * F], r_sb[0][:].bitcast(f32))
                emit_attn_tail(1, ea1)
                if s == nsteps - 1:
                    nc.sync.dma_start(qout_d[128:256, F : 2 * F], r_sb[1][:].bitcast(f32))
                if s < nsteps - 1:
                    emit_hT(r_sb, R1, R2)

            if nsteps == 0:
                for half in range(2):
                    nc.sync.dma_start(
                        qout_d[128 * half : 128 * half + 128, 0:F], h_sb[half][:].bitcast(f32)
                    )

    nc.compile()
    return nc


def _get_program(T_pad: int) -> bass.Bass:
    nsteps = int(os.environ.get("KERNEL_NSTEPS", str(STEPS)))
    key = (T_pad, nsteps)
    if key not in _PROG_CACHE:
        _PROG_CACHE[key] = _build_program(T_pad, nsteps)
    return _PROG_CACHE[key]


def make_in_maps(x, batch, cos_coef, q_star, W_ih, W_hh, b_ih, b_hh):
    """Host-side shard + re-layout. Returns (in_maps, T_pad)."""
    x = np.ascontiguousarray(np.asarray(x, dtype=np.float32))
    batch = np.asarray(batch).astype(np.int64)
    cos = np.asarray(cos_coef, dtype=np.float32)
    qs = np.asarray(q_star, dtype=np.float32)
    W_ih = np.asarray(W_ih, dtype=np.float32)
    W_hh = np.asarray(W_hh, dtype=np.float32)
    bsum = (np.asarray(b_ih, dtype=np.float32) + np.asarray(b_hh, dtype=np.float32))

    counts = np.bincount(batch, minlength=B)
    starts = np.zeros(B + 1, dtype=np.int64)
    starts[1:] = np.cumsum(counts)
    blk_counts = counts.reshape(-1, BS).sum(axis=1)
    T_pad = int(max(1, -(-blk_counts.max() // 128)))
    NT = BLOCKS * T_pad
    BW = T_pad * 128

    bf = ml_dtypes.bfloat16

    # LSTM weight stacks (fp16)
    W_ihT = W_ih.T  # [400, 800]
    W_hhT = W_hh.T  # [200, 800]
    w0 = np.concatenate(
        [W_ihT, bsum[None, :], W_hhT, np.zeros((BS + 1, 800), np.float32)], axis=0
    ).astype(np.float16)  # [634, 800]; rows 529.. = W_hhT[128:200] + aug zeros
    WcT = W_ihT[:F] + W_hhT          # [200, 800]
    WrT = W_ihT[F:]                  # [200, 800]
    wc = np.concatenate(
        [WcT[0:128], WcT[128:200], np.zeros((BS + 1, 800), np.float32),
         WrT[0:128], WrT[128:200], bsum[None, :]], axis=0
    ).astype(np.float16)             # [434, 800]

    qc2c = np.zeros((BS + 1, 256), np.float16)
    qc2c[0:BS] = np.tile(100.0 * np.eye(BS, dtype=np.float32), (1, BLOCKS))
    qc2c[BS] = -100.0

    in_maps = []
    for c in range(CORES):
        seg0 = c * SEG_PER_CORE
        xf = np.zeros((KAUG, NT * 128), dtype=np.float16)
        cwt = np.zeros((128, NT * BS), dtype=np.float16)
        xp = np.zeros((128, NT * FW), dtype=np.float16)
        for g in range(BLOCKS):
            sa = seg0 + g * BS
            n0, n1 = int(starts[sa]), int(starts[sa + BS])
            cnt = n1 - n0
            js = (batch[n0:n1] - sa).astype(np.int64)

            xb = np.zeros((BW, FW), dtype=np.float32)
            xb[:cnt, :F] = x[n0:n1]
            xb[:cnt, F] = 1.0
            xp[:, g * T_pad * FW : (g + 1) * T_pad * FW] = (
                xb.reshape(T_pad, 128, FW).transpose(1, 0, 2).reshape(128, T_pad * FW)
            ).astype(np.float16)

            xfb = np.zeros((KAUG, BW), dtype=np.float32)
            xfb[0:F, :cnt] = x[n0:n1].T
            xfb[F + js, np.arange(cnt)] = 1.0
            xfb[F + BS, :] = 1.0
            xf[:, g * BW : (g + 1) * BW] = xfb.astype(np.float16)

            wb = np.zeros((BW, BS), dtype=np.float32)
            wb[np.arange(cnt), js] = cos[n0:n1]
            cwt[:, g * T_pad * BS : (g + 1) * T_pad * BS] = (
                wb.reshape(T_pad, 128, BS).transpose(1, 0, 2).reshape(128, T_pad * BS)
            ).astype(np.float16)

        qs0t = np.ones((401, 256), dtype=np.float16)
        qs0t[0:400] = qs[seg0 : seg0 + SEG_PER_CORE].T.astype(np.float16)
        in_maps.append(
            {
                "xf1": np.ascontiguousarray(xf[0:128]),
                "xf2": np.ascontiguousarray(xf[128:KAUG]),
                "cwt": cwt,
                "xp": xp,
                "qs0t": qs0t,
                "w0": w0,
                "wc": wc,
                "qc2c": qc2c,
                "onesr": np.ones((1, 256), np.float16),
                "idf": np.eye(128, dtype=np.float32),
                "idb": np.eye(128, dtype=np.float32).astype(bf),
            }
        )
    return in_maps, T_pad


def kernel(x, batch, cos_coef, q_star, W_ih, W_hh, b_ih, b_hh):
    global LAST_RESULT
    in_maps, T_pad = make_in_maps(
        x, batch, cos_coef, q_star, W_ih, W_hh, b_ih, b_hh
    )
    nc = _get_program(T_pad)
    res = run_bass_kernel_spmd(nc, in_maps, list(range(CORES)), trace=TRACE)
    LAST_RESULT = res
    out = np.zeros((B, 2 * F), dtype=np.float32)
    for c in range(CORES):
        out[c * SEG_PER_CORE : (c + 1) * SEG_PER_CORE] = res.results[c]["qout"]
    return out
